# revision 1
# baseline (speedup 1.0000x reference)
"""Causal self-attention (B=4, T=2048, C=1024, H=16) on 8 trn2 NeuronCores.

Sharding: head-pair parallel. Core c owns heads {2c, 2c+1} for all 4 batches.
 - host: x is pre-transposed to xT [C, B*T]; W_qkv is pre-sliced per core into
   wq/wk/wv [C, 128] (2 heads x 64, softmax scale folded into wq), W_proj and
   biases broadcast.
 - device per core: qkv projections as fp32r matmuls producing qT/kT [d2, T]
   (d on partitions) and vT [d2, T]; vT is PE-transposed per 128-tile into
   v [T, 64]-per-head tiles with an appended ones column.
 - attention in S^T orientation: S^T[tk, tq] = kT.T@qT tiles [128, 512] with the
   causal mask preloaded into PSUM via an identity matmul; softmax without
   max-subtraction (|S| <= ~20, safe in fp32): P^T = exp(S^T) on ScalarE
   (PSUM->SBUF, rounded to f32r). O-matmul lhsT = [v_h | ones] (M=65) yields
   both O^T[d, tq] and the denominator row l in one pass. Normalize via
   reciprocal + K=1 broadcast matmul + DVE multiply.
 - per-batch AllToAll (1 MB/rank) reshards O^T from head-shards to
   token-shards; column-parallel out-projection with fused bias produces
   out^T [C, 1024 tokens] per core; host reassembles.
"""
import numpy as np
import concourse.bacc as bacc
import concourse.mybir as mybir
import concourse.tile as tile
from concourse.bass_utils import run_bass_kernel_spmd
from concourse.masks import make_identity

F32 = mybir.dt.float32
F32R = mybir.dt.float32r
Exp = mybir.ActivationFunctionType.Exp

NCORES = 8
B, T, C, H = 4, 2048, 1024, 16
HD = C // H          # 64
HL = H // NCORES     # 2 heads per core
D2 = HL * HD         # 128 rows of local head-pair dims
TB = T               # tokens per batch
NKC = C // 128       # 8 contraction chunks
NCH = TB // 512      # 4 tq chunks per batch
NTK = TB // 128      # 16 tk tiles per batch
PIECE = TB // NCORES  # 256 tokens per (batch, core) piece after AllToAll

_CACHE = {}


def _build(sim=False):
    nc = bacc.Bacc("TRN2", target_bir_lowering=False, debug=False,
                   num_devices=1 if sim else NCORES)
    xt = nc.dram_tensor("xt", [C, B * T], F32R, kind="ExternalInput").ap()
    wq = nc.dram_tensor("wq", [C, D2], F32R, kind="ExternalInput").ap()
    wk = nc.dram_tensor("wk", [C, D2], F32R, kind="ExternalInput").ap()
    wv = nc.dram_tensor("wv", [C, D2], F32R, kind="ExternalInput").ap()
    wp = nc.dram_tensor("wp", [C, C], F32R, kind="ExternalInput").ap()
    bqkv = nc.dram_tensor("bqkv", [D2, 3], F32, kind="ExternalInput").ap()
    bp = nc.dram_tensor("bp", [128, NKC], F32, kind="ExternalInput").ap()
    outp = nc.dram_tensor("outp", [C, B * PIECE], F32, kind="ExternalOutput").ap()

    inb = [nc.dram_tensor(f"inb{b}", [NCORES, D2, PIECE], F32R) for b in range(B)]
    outb = [nc.dram_tensor(f"outb{b}", [NCORES, D2, PIECE], F32R) for b in range(B)]

    with tile.TileContext(nc) as tc:
        with (
            tc.tile_pool(name="const", bufs=1) as cpool,
            tc.tile_pool(name="w", bufs=1) as wpool,
            tc.tile_pool(name="xt", bufs=16) as xpool,
            tc.tile_pool(name="qk", bufs=2) as qkpool,
            tc.tile_pool(name="vstg", bufs=1) as vstgpool,
            tc.tile_pool(name="vh", bufs=2) as vhpool,
            tc.tile_pool(name="pt", bufs=5) as ptpool,
            tc.tile_pool(name="small", bufs=3) as smallpool,
            tc.tile_pool(name="ofin", bufs=4) as ofinpool,
            tc.tile_pool(name="proj", bufs=3) as projpool,
            tc.tile_pool(name="otp", bufs=9) as otpool,
            tc.tile_pool(name="mm", bufs=2, space="PSUM") as mmps,
            tc.tile_pool(name="s", bufs=2, space="PSUM") as sps,
            tc.tile_pool(name="o", bufs=1, space="PSUM") as ops,
            
        ):
            # ---- constants ----
            ident32 = cpool.tile([128, 128], F32)
            make_identity(nc, ident32[:])
            idr = cpool.tile([128, 128], F32R)
            mask32 = cpool.tile([128, 512], F32)
            masks = cpool.tile([128, 4 * 512], F32R)
            ones32 = cpool.tile([128, 16], F32)
            ones64 = cpool.tile([1, 64], F32)
            onesr = cpool.tile([1, 64], F32R)
            nc.gpsimd.memset(ones32[:], 1.0)
            nc.gpsimd.memset(ones64[:], 1.0)
            with nc.allow_low_precision(reason="f32r operand staging"):
                nc.vector.tensor_copy(idr[:], ident32[:])
                nc.vector.tensor_copy(onesr[:], ones64[:])
                for m in range(4):
                    nc.gpsimd.memset(mask32[:], 0.0)
                    # keep where tq_local >= tk_local + 128*m
                    nc.gpsimd.affine_select(
                        out=mask32[:], in_=mask32[:],
                        compare_op=mybir.AluOpType.is_ge, fill=-1e30,
                        base=-128 * m, channel_multiplier=-1,
                        pattern=[[1, 512]],
                    )
                    nc.vector.tensor_copy(masks[:, 512 * m:512 * (m + 1)],
                                          mask32[:])

            # ---- weights ----
            wq_sb = wpool.tile([128, NKC, D2], F32R)
            wk_sb = wpool.tile([128, NKC, D2], F32R)
            wv_sb = wpool.tile([128, NKC, D2], F32R)
            for t, d in ((wq_sb, wq), (wk_sb, wk), (wv_sb, wv)):
                nc.sync.dma_start(
                    t[:], d.rearrange("(kc p) m -> p kc m", p=128))
            wp_sb = wpool.tile([128, NKC, C], F32R)
            nc.sync.dma_start(
                wp_sb[:], wp.rearrange("(kc p) m -> p kc m", p=128))
            bqkv_sb = cpool.tile([D2, 3], F32)
            nc.sync.dma_start(bqkv_sb[:], bqkv)
            bp_sb = cpool.tile([128, NKC], F32)
            nc.sync.dma_start(bp_sb[:], bp)

            for b in range(B):
                g0 = b * TB
                # ---- qkv projections ----
                qT = qkpool.tile([D2, TB], F32R, tag="qT")
                kT = qkpool.tile([D2, TB], F32R, tag="kT")
                vT = vstgpool.tile([D2, TB], F32)
                for n in range(NCH):
                    xts = []
                    for kc in range(NKC):
                        xtile = xpool.tile([128, 512], F32R)
                        nc.sync.dma_start(
                            xtile[:],
                            xt[128 * kc:128 * (kc + 1),
                               g0 + 512 * n:g0 + 512 * (n + 1)])
                        xts.append(xtile)
                    for w_sb, col in ((wq_sb, 0), (wk_sb, 1), (wv_sb, 2)):
                        ps = mmps.tile([128, 512], F32, tag="ps")
                        for kc in range(NKC):
                            nc.tensor.matmul(
                                ps[:], w_sb[:, kc, :],
                                xts[kc][:], start=(kc == 0),
                                stop=(kc == NKC - 1))
                        dst = (qT, kT, vT)[col]
                        with nc.allow_low_precision(reason="f32r qkv"):
                            nc.vector.tensor_scalar_add(
                                dst[:, 512 * n:512 * (n + 1)], ps[:],
                                bqkv_sb[:, col:col + 1])

                # ---- v transposes: vT [d2, T] -> per-head v [T, 65] tiles ----
                vh = [vhpool.tile([128, NTK * 65], F32R, tag=f"vh{h}",
                                  name=f"vh{h}") for h in range(HL)]
                for h in range(HL):
                    with nc.allow_low_precision(reason="f32r v ones"):
                        nc.vector.tensor_copy(vh[h][:, 64::65], ones32[:])
                    for tk in range(NTK):
                        vt_ps = mmps.tile([128, 64], F32, tag="ps", name="vt_ps")
                        nc.tensor.transpose(
                            vt_ps[:],
                            vT[64 * h:64 * (h + 1), 128 * tk:128 * (tk + 1)],
                            ident32[64 * h:64 * (h + 1), 64 * h:64 * (h + 1)])
                        with nc.allow_low_precision(reason="f32r v"):
                            nc.vector.tensor_copy(
                                vh[h][:, 65 * tk:65 * tk + 64], vt_ps[:])

                # ---- attention per tq-chunk ----
                for j in range(NCH):
                    o_ps = [ops.tile([65, 512], F32, tag=f"o{h}", name=f"o{h}")
                            for h in range(HL)]
                    ktop = 4 * j + 4
                    for tk in range(ktop):
                        m = tk - 4 * j
                        # cols [0, z) of this tile are fully causal-masked
                        z = 128 * m if m > 0 else 0
                        w = 512 - z
                        s_ps = sps.tile([128, 1024], F32, tag="s_ps")
                        if m >= 0:
                            for h in range(HL):
                                nc.tensor.matmul(
                                    s_ps[:, 512 * h + z:512 * (h + 1)],
                                    idr[:],
                                    masks[:, 512 * m + z:512 * (m + 1)],
                                    start=True, stop=False)
                        # K=64 pair at row groups (0,0)/(64,0) -> concurrent
                        for h in range(HL):
                            nc.tensor.matmul(
                                s_ps[:, 512 * h + z:512 * (h + 1)],
                                kT[64 * h:64 * (h + 1),
                                   128 * tk:128 * (tk + 1)],
                                qT[64 * h:64 * (h + 1),
                                   512 * j + z:512 * (j + 1)],
                                start=(m < 0), stop=True)
                        pt = ptpool.tile([128, 1024], F32R, tag="pt")
                        if z:
                            exp_src = s_ps[:].rearrange(
                                "p (g c) -> p g c", g=2)[:, :, z:]
                            exp_dst = pt[:].rearrange(
                                "p (g c) -> p g c", g=2)[:, :, z:]
                            nc.scalar.activation(exp_dst, exp_src, Exp)
                        else:
                            nc.scalar.activation(pt[:], s_ps[:], Exp)
                        for h in range(HL):
                            nc.tensor.matmul(
                                o_ps[h][0:65, z:512],
                                vh[h][:, 65 * tk:65 * (tk + 1)],
                                pt[:, 512 * h + z:512 * (h + 1)],
                                start=(tk == 0), stop=(tk == ktop - 1))
                    for h in range(HL):
                        o_sb = smallpool.tile([65, 512], F32, tag="osb2")
                        nc.vector.tensor_copy(o_sb[:], o_ps[h][:])
                        r_sb = smallpool.tile([1, 512], F32R, tag="r")
                        with nc.allow_low_precision(reason="softmax denom"):
                            nc.vector.reciprocal(r_sb[:], o_sb[64:65, :])
                        rb_ps = mmps.tile([64, 512], F32, tag="ps", name="rb_ps")
                        nc.tensor.matmul(rb_ps[:], onesr[:], r_sb[:],
                                         start=True, stop=True)
                        rb_sb = smallpool.tile([64, 512], F32, tag="rb")
                        nc.vector.tensor_copy(rb_sb[:], rb_ps[:])
                        ofin = ofinpool.tile([64, 512], F32R)
                        with nc.allow_low_precision(reason="f32r O"):
                            nc.gpsimd.tensor_mul(ofin[:], o_sb[0:64, :],
                                                 rb_sb[:])
                        for half in range(2):
                            s8 = 2 * j + half
                            nc.sync.dma_start(
                                inb[b].ap()[s8, 64 * h:64 * (h + 1), :],
                                ofin[:, 256 * half:256 * (half + 1)])

                # ---- AllToAll: head-shards -> token-shards ----
                if sim:
                    # stand-in with comparable cost for the cost-model sim
                    nc.sync.dma_start(outb[b].ap(), inb[b].ap())
                else:
                    nc.gpsimd.collective_compute(
                        "AllToAll", mybir.AluOpType.bypass,
                        replica_groups=[list(range(NCORES))],
                        ins=[inb[b].ap().opt()], outs=[outb[b].ap().opt()],
                    )

                # ---- out projection (column-parallel, out^T) ----
                ots = []
                for s8 in range(NCORES):
                    ot = otpool.tile([128, PIECE], F32R, tag="ot")
                    nc.sync.dma_start(ot[:], outb[b].ap()[s8])
                    ots.append(ot)
                for mcol in range(NKC):
                    pp = mmps.tile([128, PIECE], F32, tag="ps")
                    for s8 in range(NCORES):
                        nc.tensor.matmul(
                            pp[:],
                            wp_sb[:, s8, 128 * mcol:128 * (mcol + 1)],
                            ots[s8][:], start=(s8 == 0),
                            stop=(s8 == NCORES - 1))
                    osb = projpool.tile([128, PIECE], F32, tag="osb")
                    nc.vector.tensor_scalar_add(osb[:], pp[:],
                                                bp_sb[:, mcol:mcol + 1])
                    nc.sync.dma_start(
                        outp[128 * mcol:128 * (mcol + 1),
                             PIECE * b:PIECE * (b + 1)], osb[:])
    nc.compile()
    return nc


def _get_nc():
    if "nc" not in _CACHE:
        _CACHE["nc"] = _build()
    return _CACHE["nc"]


def kernel(x, W_qkv, b_qkv, W_proj, b_proj):
    x = np.asarray(x, dtype=np.float32)
    W_qkv = np.asarray(W_qkv, dtype=np.float32)
    b_qkv = np.asarray(b_qkv, dtype=np.float32)
    W_proj = np.asarray(W_proj, dtype=np.float32)
    b_proj = np.asarray(b_proj, dtype=np.float32)

    scale = 1.0 / np.sqrt(HD)
    xt = np.ascontiguousarray(x.reshape(B * T, C).T)          # [C, B*T]
    wp = np.ascontiguousarray(W_proj)                          # [C, C]
    bp = np.ascontiguousarray(b_proj.reshape(NKC, 128).T)      # [128, 8]

    qw = W_qkv[:, 0:C]
    kw = W_qkv[:, C:2 * C]
    vw = W_qkv[:, 2 * C:3 * C]
    qb, kb, vb = b_qkv[0:C], b_qkv[C:2 * C], b_qkv[2 * C:3 * C]

    in_maps = []
    for c in range(NCORES):
        cols = slice(2 * c * HD, (2 * c + 2) * HD)  # this core's 128 dims
        bq = np.stack([qb[cols] * scale, kb[cols], vb[cols]], axis=1)  # [128,3]
        in_maps.append({
            "xt": xt,
            "wq": np.ascontiguousarray(qw[:, cols] * scale),
            "wk": np.ascontiguousarray(kw[:, cols]),
            "wv": np.ascontiguousarray(vw[:, cols]),
            "wp": wp,
            "bqkv": np.ascontiguousarray(bq),
            "bp": bp,
        })

    nc = _get_nc()
    _CACHE["last_in_maps"] = in_maps
    res = run_bass_kernel_spmd(nc, in_maps, core_ids=list(range(NCORES)))

    # outp[c]: [C, B*PIECE] (cols: b-major, then 256 tokens of piece c)
    allo = np.stack([res.results[c]["outp"] for c in range(NCORES)])
    allo = allo.reshape(NCORES, C, B, PIECE)       # [c, ch, b, u]
    out = allo.transpose(2, 0, 3, 1).reshape(B, T, C)
    return np.ascontiguousarray(out)



# revision 50
# speedup vs baseline: 1.5821x; 1.5821x over previous
"""Causal self-attention (B=4, T=2048, C=1024, H=16) on 8 trn2 NeuronCores.

Sharding: head-pair parallel. Core c owns heads {2c, 2c+1} for all 4 batches.
 - host: x is pre-transposed to xT [C, B*T] in bf16; W_qkv is pre-sliced per
   core into wq/wk/wv [C, 128] bf16 (2 heads x 64, softmax scale folded into
   wq), W_proj bf16 and f32 biases broadcast.
 - device per core: qkv projections as bf16 matmuls producing qT/kT/vT
   [d2, T] bf16 (d on partitions); vT is PE-transposed per 128-tile into a
   combined per-head v [T, 65] layout (ones column appended for the softmax
   denominator).
 - attention in S^T orientation: S^T[tk, tq] = kT.T@qT tiles [128, 512], both
   heads side by side in one PSUM tile [128, 1024]. Softmax without
   max-subtraction (|S| <= ~20, safe in f32 PSUM): P^T = exp(S^T) on ScalarE
   (PSUM->SBUF, bf16). Causal masking of the diagonal 128x128 sub-block is a
   post-exp affine_select (fill 0) on Pool; fully-masked columns are skipped
   via the z offset. O-matmul lhsT = [v_h | ones] (M=65) yields both
   O^T[d, tq] and the denominator row l in one pass. Normalize via
   reciprocal + two K=1 broadcast matmuls + DVE multiply into a per-batch
   O^T accumulator [128, T] bf16.
 - resharding in 64-token strips: rank r's piece of tq-chunk j is tokens
   [512j + 64r, 512j + 64r + 64), so each chunk's AllToAll (128 KB) fires as
   soon as that chunk's softmax completes and the received strips accumulate
   into a per-batch ot tile; the column-parallel out-projection with fused
   bias is deferred two batches and produces out^T bf16; host reassembles.
 - scheduling: all non-attention PE work (qkv chains of later batches, v
   transposes, deferred out-projections) is chopped into ~0.2-0.4us filler
   units in a global queue and emitted between attention tiles under a
   PE-nanosecond budget, so the PE never idles on the exp round-trip or the
   collectives; per-chunk marks force-drain a batch's qkv units before the
   attention that reads them (the in-order PE queue must never wait on work
   emitted after it). S->exp->O is software-pipelined (O lags S by 3 tiles).
"""
import numpy as np
import ml_dtypes
import concourse.bacc as bacc
import concourse.mybir as mybir
import concourse.tile as tile
from concourse.bass_utils import run_bass_kernel_spmd
from concourse.masks import make_identity

F32 = mybir.dt.float32
F32R = mybir.dt.float32r
BF = mybir.dt.bfloat16
Exp = mybir.ActivationFunctionType.Exp

NCORES = 8
B, T, C, H = 4, 2048, 1024, 16
HD = C // H          # 64
HL = H // NCORES     # 2 heads per core
D2 = HL * HD         # 128 rows of local head-pair dims
TB = T               # tokens per batch
NKC = C // 128       # 8 contraction chunks
NCH = TB // 512      # 4 tq chunks per batch
NTK = TB // 128      # 16 tk tiles per batch
PIECE = TB // NCORES  # 256 tokens per (batch, core) piece after AllToAll

_CACHE = {}


def _build(sim=False):
    nc = bacc.Bacc("TRN2", target_bir_lowering=False, debug=False,
                   num_devices=1 if sim else NCORES)
    xt = nc.dram_tensor("xt", [C, B * T], BF, kind="ExternalInput").ap()
    wq = nc.dram_tensor("wq", [C, D2], BF, kind="ExternalInput").ap()
    wk = nc.dram_tensor("wk", [C, D2], BF, kind="ExternalInput").ap()
    wv = nc.dram_tensor("wv", [C, D2], BF, kind="ExternalInput").ap()
    wp = nc.dram_tensor("wp", [C, C], BF, kind="ExternalInput").ap()
    bqkv = nc.dram_tensor("bqkv", [D2, 3], F32, kind="ExternalInput").ap()
    bp = nc.dram_tensor("bp", [128, NKC], F32, kind="ExternalInput").ap()
    outp = nc.dram_tensor("outp", [C, B * PIECE], BF,
                          kind="ExternalOutput").ap()

    # per-(batch, j-chunk) 64-token strips: rank r's piece of chunk j is
    # tokens [512j + 64r, 512j + 64r + 64) — uniform across ranks, so each
    # chunk's AllToAll can fire as soon as that chunk's softmax completes.
    inb = [[nc.dram_tensor(f"inb{b}_{j}", [NCORES, D2, 64], BF)
            for j in range(NCH)] for b in range(B)]
    outb = [[nc.dram_tensor(f"outb{b}_{j}", [NCORES, D2, 64], BF)
             for j in range(NCH)] for b in range(B)]

    xt_r = xt.rearrange("(kc p) t -> p kc t", p=128)
    outp_r = outp.rearrange("(kc p) t -> p kc t", p=128)

    with tile.TileContext(nc) as tc:
        with (
            tc.tile_pool(name="const", bufs=1) as cpool,
            tc.tile_pool(name="w", bufs=1) as wpool,
            tc.tile_pool(name="xb", bufs=2) as xpool,
            tc.tile_pool(name="qk", bufs=2) as qkpool,
            tc.tile_pool(name="vstg", bufs=2) as vstgpool,
            tc.tile_pool(name="vh", bufs=2) as vhpool,
            tc.tile_pool(name="pt", bufs=5) as ptpool,
            tc.tile_pool(name="small", bufs=2) as smallpool,
            tc.tile_pool(name="obuf", bufs=2) as obufpool,
            tc.tile_pool(name="proj", bufs=4) as projpool,
            tc.tile_pool(name="otp", bufs=4) as otpool,
            tc.tile_pool(name="wps", bufs=2, space="PSUM") as wps,
            tc.tile_pool(name="s", bufs=2, space="PSUM") as sps,
            tc.tile_pool(name="o", bufs=1, space="PSUM") as ops,
        ):
            # ---- constants ----
            ident32 = cpool.tile([128, 128], F32)
            make_identity(nc, ident32[:])
            identb = cpool.tile([128, 128], BF)
            onesb = cpool.tile([128, HL * NTK], BF)
            # head-selector rows for the denominator broadcast matmuls;
            # single-partition tiles (partition starts must be 0/32/64/96)
            sel01 = cpool.tile([1, 256], BF)
            nc.gpsimd.memset(onesb[:], 1.0)
            nc.gpsimd.memset(sel01[:], 0.0)
            nc.gpsimd.memset(sel01[0:1, 0:64], 1.0)
            nc.gpsimd.memset(sel01[0:1, 192:256], 1.0)
            with nc.allow_low_precision(reason="bf16 operand staging"):
                nc.vector.tensor_copy(identb[:], ident32[:])

            # ---- state ----
            st = {}

            def load_xb_chunk(b, n):
                if ("xb", b) not in st:
                    st["xb", b] = xpool.tile([128, NKC, TB], BF, tag="xb",
                                             name=f"xb{b}")
                nc.sync.dma_start(
                    st["xb", b][:, :, 512 * n:512 * (n + 1)],
                    xt_r[:, :, TB * b + 512 * n:TB * b + 512 * (n + 1)])

            # ---- input + weight DMAs, most-urgent first ----
            # wq first, then the first chunk in 2-kc pieces so the first
            # qkv chain units start as soon as their slices land
            st["xb", 0] = xpool.tile([128, NKC, TB], BF, tag="xb",
                                     name="xb0")
            wq_sb = wpool.tile([128, NKC, D2], BF)
            wk_sb = wpool.tile([128, NKC, D2], BF)
            wv_sb = wpool.tile([128, NKC, D2], BF)
            nc.sync.dma_start(
                wq_sb[:], wq.rearrange("(kc p) m -> p kc m", p=128))
            for kc0 in range(0, NKC, 2):
                nc.sync.dma_start(
                    st["xb", 0][:, kc0:kc0 + 2, 0:512],
                    xt_r[:, kc0:kc0 + 2, 0:512])
            nc.sync.dma_start(
                wk_sb[:], wk.rearrange("(kc p) m -> p kc m", p=128))
            nc.sync.dma_start(
                wv_sb[:], wv.rearrange("(kc p) m -> p kc m", p=128))
            bqkv_sb = cpool.tile([D2, 3], F32)
            nc.sync.dma_start(bqkv_sb[:], bqkv)
            for n in range(1, NCH):
                load_xb_chunk(0, n)
            wp_sb = wpool.tile([128, NKC, C], BF)
            nc.sync.dma_start(
                wp_sb[:], wp.rearrange("(kc p) m -> p kc m", p=128))
            bp_sb = cpool.tile([128, NKC], F32)
            nc.sync.dma_start(bp_sb[:], bp)

            # ---- building blocks ----
            def qkv_alloc(b):
                st["qT", b] = qkpool.tile([D2, TB], BF, tag="qT",
                                          name=f"qT{b}")
                st["kT", b] = qkpool.tile([D2, TB], BF, tag="kT",
                                          name=f"kT{b}")
                st["vT", b] = vstgpool.tile([D2, TB], BF, tag="vT",
                                            name=f"vT{b}")

            def vh_alloc(b):
                vh = vhpool.tile([128, HL, NTK * 65], BF, tag="vh",
                                 name=f"vh{b}")
                st["vh", b] = vh
                with nc.allow_low_precision(reason="bf16 v ones"):
                    nc.vector.tensor_copy(
                        vh[:, :, 64::65],
                        onesb[:].rearrange("p (g t) -> p g t", g=HL))

            def qkv_chain_units(b, n, col):
                """Units (2 matmuls each) for one qkv projection chain."""
                w_sb = (wq_sb, wk_sb, wv_sb)[col]
                hold = {}

                def unit(kc0, b=b, n=n, col=col):
                    def f():
                        if kc0 == 0:
                            if n == 0 and col == 0:
                                qkv_alloc(b)
                            hold["ps"] = wps.tile([128, 512], F32, tag="w",
                                                  name=f"qkvps{b}_{n}_{col}")
                        ps = hold["ps"]
                        xbt = st["xb", b]
                        for kc in (kc0, kc0 + 1):
                            nc.tensor.matmul(
                                ps[:], w_sb[:, kc, :],
                                xbt[:, kc, 512 * n:512 * (n + 1)],
                                start=(kc == 0), stop=(kc == NKC - 1))
                        if kc0 == NKC - 2:
                            dst = (st["qT", b], st["kT", b],
                                   st["vT", b])[col]
                            with nc.allow_low_precision(reason="bf16 qkv"):
                                nc.vector.tensor_scalar_add(
                                    dst[:, 512 * n:512 * (n + 1)], ps[:],
                                    bqkv_sb[:, col:col + 1])
                    return f
                return [(unit(kc0), 426) for kc0 in range(0, NKC, 2)]

            def vtrans_unit(b, tk):
                def f():
                    if tk == 0:
                        vh_alloc(b)
                    vh, vT = st["vh", b], st["vT", b]
                    vt_ps = wps.tile([128, 128], BF, tag="w",
                                     name=f"vt{b}_{tk}")
                    nc.tensor.transpose(
                        vt_ps[:], vT[:, 128 * tk:128 * (tk + 1)], identb[:])
                    with nc.allow_low_precision(reason="bf16 v"):
                        nc.vector.tensor_copy(
                            vh[:, :, 65 * tk:65 * tk + 64],
                            vt_ps[:].rearrange("p (g d) -> p g d", g=HL))
                return f, 60

            def qkv_chunk_units(b, n):
                """All units for qkv chunk n of batch b (q,k,v chains; the
                v chain is followed by the transposes it unlocks)."""
                units = []
                for col in range(3):
                    units += qkv_chain_units(b, n, col)
                for tk in range(4 * n, 4 * n + 4):
                    units.append(vtrans_unit(b, tk))
                return units

            def proj_units(bb):
                """Out-projection of batch bb, one unit = 2 matmuls.
                Reads the per-j ot strips loaded during attn(bb)."""
                hold = {}

                def unit(mcol, s0, bb=bb):
                    def f():
                        if mcol == 0 and s0 == 0:
                            hold["psb"] = projpool.tile(
                                [128, NKC, PIECE], BF, tag="psb",
                                name=f"psb{bb}")
                        if s0 == 0:
                            hold["pp"] = wps.tile([128, PIECE], F32, tag="w",
                                                  name=f"pp{bb}_{mcol}")
                        pp, ot = hold["pp"], st["ot", bb]
                        for s8 in (s0, s0 + 1):
                            nc.tensor.matmul(
                                pp[:],
                                wp_sb[:, s8, 128 * mcol:128 * (mcol + 1)],
                                ot[:, s8], start=(s8 == 0),
                                stop=(s8 == NCORES - 1))
                        if s0 == NCORES - 2:
                            with nc.allow_low_precision(reason="bf16 out"):
                                nc.vector.tensor_scalar_add(
                                    hold["psb"][:, mcol, :], pp[:],
                                    bp_sb[:, mcol:mcol + 1])
                            if mcol == 5:
                                nc.sync.dma_start(
                                    outp_r[:, 0:6,
                                           PIECE * bb:PIECE * (bb + 1)],
                                    hold["psb"][:, 0:6, :])
                            if mcol == NKC - 1:
                                nc.sync.dma_start(
                                    outp_r[:, 6:8,
                                           PIECE * bb:PIECE * (bb + 1)],
                                    hold["psb"][:, 6:8, :])
                    return f
                return [(unit(mcol, s0), 214) for mcol in range(NKC)
                        for s0 in range(0, NCORES, 2)]

            class Fill:
                """Global queue of small PE work units. Units are emitted
                into the instruction stream between attention tiles so the
                PE never idles on the exp round-trip. Marks record queue
                positions that must be fully emitted before a dependent
                attention chunk runs (deadlock avoidance: the in-order PE
                queue may never wait on work emitted after it)."""

                def __init__(self):
                    self.q = []
                    self.i = 0
                    self.marks = {}

                def add(self, units, mark=None):
                    self.q += units
                    if mark is not None:
                        self.marks[mark] = len(self.q)

                def run_ns(self, budget):
                    while budget > 0 and self.i < len(self.q):
                        fn, est = self.q[self.i]
                        fn()
                        self.i += 1
                        budget -= est

                def drain_to(self, mark):
                    upto = self.marks.get(mark, 0)
                    while self.i < upto:
                        self.q[self.i][0]()
                        self.i += 1

                def drain(self):
                    while self.i < len(self.q):
                        self.q[self.i][0]()
                        self.i += 1

            def attn_j(b, j, fill, tile_fill):
                fill.drain_to(("qkv", b, j))
                qT, kT, vh = st["qT", b], st["kT", b], st["vh", b]
                obuf = st["obuf", b]
                if j == 0:
                    st["ot", b] = otpool.tile([128, NCORES, NCH, 64], BF,
                                              tag="ot", name=f"ot{b}")
                o_ps = ops.tile([65, 1024], F32, tag="o", name=f"o{b}_{j}")
                ktop = 4 * j + 4
                pend = []          # tks with emitted exp but not yet O

                def emit_o(tk):
                    m = tk - 4 * j
                    z = 128 * m if m > 0 else 0
                    pt = st["pt", tk]
                    for h in range(HL):
                        nc.tensor.matmul(
                            o_ps[0:65, 512 * h + z:512 * (h + 1)],
                            vh[:, h, 65 * tk:65 * (tk + 1)],
                            pt[:, 512 * h + z:512 * (h + 1)],
                            start=(tk == 0), stop=(tk == ktop - 1))

                for tk in range(ktop):
                    m = tk - 4 * j
                    z = 128 * m if m > 0 else 0
                    s_ps = sps.tile([128, 1024], F32, tag="s_ps")
                    for h in range(HL):
                        nc.tensor.matmul(
                            s_ps[:, 512 * h + z:512 * (h + 1)],
                            kT[64 * h:64 * (h + 1),
                               128 * tk:128 * (tk + 1)],
                            qT[64 * h:64 * (h + 1),
                               512 * j + z:512 * (j + 1)],
                            start=True, stop=True)
                    pt = ptpool.tile([128, 1024], BF, tag="pt")
                    st["pt", tk] = pt
                    with nc.allow_low_precision(reason="bf16 P"):
                        if z:
                            exp_src = s_ps[:].rearrange(
                                "p (g c) -> p g c", g=2)[:, :, z:]
                            exp_dst = pt[:].rearrange(
                                "p (g c) -> p g c", g=2)[:, :, z:]
                            nc.scalar.activation(exp_dst, exp_src, Exp)
                        else:
                            nc.scalar.activation(pt[:], s_ps[:], Exp)
                        if m >= 0:
                            # zero the upper triangle of the mixed 128-block
                            blk = pt[:].rearrange(
                                "p (g c) -> p g c", g=2)[:, :, z:z + 128]
                            nc.gpsimd.affine_select(
                                out=blk, in_=blk,
                                compare_op=mybir.AluOpType.is_ge, fill=0.0,
                                base=0, channel_multiplier=-1,
                                pattern=[[0, 2], [1, 128]],
                            )
                    fill.run_ns(tile_fill)
                    pend.append(tk)
                    if len(pend) > 3:
                        emit_o(pend.pop(0))
                for tk in pend:
                    emit_o(tk)

                # ---- normalization (both heads at once) ----
                r1 = smallpool.tile([1, 1024], BF, tag="r1")
                with nc.allow_low_precision(reason="softmax denom"):
                    for h in range(HL):
                        nc.vector.reciprocal(
                            r1[0:1, 512 * h:512 * h + 512],
                            o_ps[64:65, 512 * h:512 * h + 512])
                fill.run_ns(1200)
                rb_ps = wps.tile([128, 512], F32, tag="w", name=f"rb{b}_{j}")
                nc.tensor.matmul(rb_ps[:], sel01[0:1, 0:128],
                                 r1[0:1, 0:512], start=True, stop=False)
                nc.tensor.matmul(rb_ps[:], sel01[0:1, 128:256],
                                 r1[0:1, 512:1024], start=False, stop=True)
                rb_sb = smallpool.tile([128, 512], F32, tag="rb")
                nc.vector.tensor_copy(rb_sb[:], rb_ps[:])
                fill.run_ns(600)
                with nc.allow_low_precision(reason="bf16 O"):
                    for h in range(HL):
                        nc.vector.tensor_mul(
                            obuf[64 * h:64 * (h + 1),
                                 512 * j:512 * (j + 1)],
                            o_ps[0:64, 512 * h:512 * h + 512],
                            rb_sb[64 * h:64 * (h + 1), :])
                # ship this j-chunk: its 64-token strips are final, so the
                # AllToAll for this chunk fires immediately; the received
                # strips land in this batch's ot tile for the deferred proj.
                nc.sync.dma_start(
                    inb[b][j].ap().rearrange("u d p -> d u p"),
                    obuf[:, 512 * j:512 * (j + 1)].rearrange(
                        "d (u p) -> d u p", u=NCORES))
                if sim:
                    # stand-in with comparable cost for the cost-model sim
                    nc.sync.dma_start(outb[b][j].ap(), inb[b][j].ap())
                else:
                    nc.gpsimd.collective_compute(
                        "AllToAll", mybir.AluOpType.bypass,
                        replica_groups=[list(range(NCORES))],
                        ins=[inb[b][j].ap().opt()],
                        outs=[outb[b][j].ap().opt()],
                    )
                nc.sync.dma_start(
                    st["ot", b][:, :, j, :],
                    outb[b][j].ap().rearrange("u d p -> d u p"))

            # ---- main schedule ----
            # batch 0 prelude: chunk 0 of qkv(0) emitted directly
            for u, _ in qkv_chunk_units(0, 0):
                u()
            fill = Fill()
            for b in range(B):
                st["obuf", b] = obufpool.tile([128, TB], BF, tag="obuf",
                                              name=f"obuf{b}")
                if b == 0:
                    for n in range(1, NCH):
                        fill.add(qkv_chunk_units(0, n), mark=("qkv", 0, n))
                if b >= 2:
                    fill.add(proj_units(b - 2))
                if b + 1 < B:
                    for n in range(NCH):
                        load_xb_chunk(b + 1, n)
                    for n in range(NCH):
                        fill.add(qkv_chunk_units(b + 1, n),
                                 mark=("qkv", b + 1, n))
                for j in range(NCH):
                    attn_j(b, j, fill, 400)
            fill.add(proj_units(B - 2))
            fill.add(proj_units(B - 1))
            fill.drain()
    nc.compile()
    return nc


def _get_nc():
    if "nc" not in _CACHE:
        _CACHE["nc"] = _build()
    return _CACHE["nc"]


def kernel(x, W_qkv, b_qkv, W_proj, b_proj):
    x = np.asarray(x, dtype=np.float32)
    W_qkv = np.asarray(W_qkv, dtype=np.float32)
    b_qkv = np.asarray(b_qkv, dtype=np.float32)
    W_proj = np.asarray(W_proj, dtype=np.float32)
    b_proj = np.asarray(b_proj, dtype=np.float32)

    bf = ml_dtypes.bfloat16
    scale = 1.0 / np.sqrt(HD)
    xt = np.ascontiguousarray(x.reshape(B * T, C).T.astype(bf))  # [C, B*T]
    wp = np.ascontiguousarray(W_proj.astype(bf))                 # [C, C]
    bp = np.ascontiguousarray(b_proj.reshape(NKC, 128).T)        # [128, 8]

    qw = W_qkv[:, 0:C]
    kw = W_qkv[:, C:2 * C]
    vw = W_qkv[:, 2 * C:3 * C]
    qb, kb, vb = b_qkv[0:C], b_qkv[C:2 * C], b_qkv[2 * C:3 * C]

    in_maps = []
    for c in range(NCORES):
        cols = slice(2 * c * HD, (2 * c + 2) * HD)  # this core's 128 dims
        bq = np.stack([qb[cols] * scale, kb[cols], vb[cols]], axis=1)  # [128,3]
        in_maps.append({
            "xt": xt,
            "wq": np.ascontiguousarray((qw[:, cols] * scale).astype(bf)),
            "wk": np.ascontiguousarray(kw[:, cols].astype(bf)),
            "wv": np.ascontiguousarray(vw[:, cols].astype(bf)),
            "wp": wp,
            "bqkv": np.ascontiguousarray(bq, dtype=np.float32),
            "bp": np.ascontiguousarray(bp, dtype=np.float32),
        })

    nc = _get_nc()
    _CACHE["last_in_maps"] = in_maps
    res = run_bass_kernel_spmd(nc, in_maps, core_ids=list(range(NCORES)))

    # outp[r]: [C, B*PIECE] (cols: b-major, then rank r's 4 strips of 64
    # tokens, one per 512-token j-chunk: global token = 512j + 64r + t)
    allo = np.stack([np.asarray(res.results[c]["outp"])
                     for c in range(NCORES)])
    allo = allo.reshape(NCORES, C, B, NCH, 64).astype(np.float32)
    out = allo.transpose(2, 3, 0, 4, 1).reshape(B, T, C)
    return np.ascontiguousarray(out)


# revision 51
# speedup vs baseline: 1.5822x; 1.0001x over previous
"""Causal self-attention (B=4, T=2048, C=1024, H=16) on 8 trn2 NeuronCores.

Sharding: head-pair parallel. Core c owns heads {2c, 2c+1} for all 4 batches.
 - host: x is pre-transposed to xT [C, B*T] in bf16; W_qkv is pre-sliced per
   core into wq/wk/wv [C, 128] bf16 (2 heads x 64, softmax scale folded into
   wq), W_proj bf16 and f32 biases broadcast.
 - device per core: qkv projections as bf16 matmuls producing qT/kT/vT
   [d2, T] bf16 (d on partitions); vT is PE-transposed per 128-tile into a
   combined per-head v [T, 65] layout (ones column appended for the softmax
   denominator).
 - attention in S^T orientation: S^T[tk, tq] = kT.T@qT tiles [128, 512], both
   heads side by side in one PSUM tile [128, 1024]. Softmax without
   max-subtraction (|S| <= ~20, safe in f32 PSUM): P^T = exp(S^T) on ScalarE
   (PSUM->SBUF, bf16). Causal masking of the diagonal 128x128 sub-block is a
   post-exp affine_select (fill 0) on Pool; fully-masked columns are skipped
   via the z offset. O-matmul lhsT = [v_h | ones] (M=65) yields both
   O^T[d, tq] and the denominator row l in one pass. Normalize via
   reciprocal + two K=1 broadcast matmuls + DVE multiply into a per-batch
   O^T accumulator [128, T] bf16.
 - resharding in 64-token strips: rank r's piece of tq-chunk j is tokens
   [512j + 64r, 512j + 64r + 64), so each chunk's AllToAll (128 KB) fires as
   soon as that chunk's softmax completes and the received strips accumulate
   into a per-batch ot tile; the column-parallel out-projection with fused
   bias is deferred two batches and produces out^T bf16; host reassembles.
 - scheduling: all non-attention PE work (qkv chains of later batches, v
   transposes, deferred out-projections) is chopped into ~0.2-0.4us filler
   units in a global queue and emitted between attention tiles under a
   PE-nanosecond budget, so the PE never idles on the exp round-trip or the
   collectives; per-chunk marks force-drain a batch's qkv units before the
   attention that reads them (the in-order PE queue must never wait on work
   emitted after it). S->exp->O is software-pipelined (O lags S by 3 tiles).
"""
import numpy as np
import ml_dtypes
import concourse.bacc as bacc
import concourse.mybir as mybir
import concourse.tile as tile
from concourse.bass_utils import run_bass_kernel_spmd
from concourse.masks import make_identity

F32 = mybir.dt.float32
F32R = mybir.dt.float32r
BF = mybir.dt.bfloat16
Exp = mybir.ActivationFunctionType.Exp

NCORES = 8
B, T, C, H = 4, 2048, 1024, 16
HD = C // H          # 64
HL = H // NCORES     # 2 heads per core
D2 = HL * HD         # 128 rows of local head-pair dims
TB = T               # tokens per batch
NKC = C // 128       # 8 contraction chunks
NCH = TB // 512      # 4 tq chunks per batch
NTK = TB // 128      # 16 tk tiles per batch
PIECE = TB // NCORES  # 256 tokens per (batch, core) piece after AllToAll

_CACHE = {}


def _build(sim=False):
    nc = bacc.Bacc("TRN2", target_bir_lowering=False, debug=False,
                   num_devices=1 if sim else NCORES)
    xt = nc.dram_tensor("xt", [C, B * T], BF, kind="ExternalInput").ap()
    wq = nc.dram_tensor("wq", [C, D2], BF, kind="ExternalInput").ap()
    wk = nc.dram_tensor("wk", [C, D2], BF, kind="ExternalInput").ap()
    wv = nc.dram_tensor("wv", [C, D2], BF, kind="ExternalInput").ap()
    wp = nc.dram_tensor("wp", [C, C], BF, kind="ExternalInput").ap()
    bqkv = nc.dram_tensor("bqkv", [D2, 3], F32, kind="ExternalInput").ap()
    bp = nc.dram_tensor("bp", [128, NKC], F32, kind="ExternalInput").ap()
    outp = nc.dram_tensor("outp", [C, B * PIECE], BF,
                          kind="ExternalOutput").ap()

    # per-(batch, j-chunk) 64-token strips: rank r's piece of chunk j is
    # tokens [512j + 64r, 512j + 64r + 64) — uniform across ranks, so each
    # chunk's AllToAll can fire as soon as that chunk's softmax completes.
    inb = [[nc.dram_tensor(f"inb{b}_{j}", [NCORES, D2, 64], BF)
            for j in range(NCH)] for b in range(B)]
    outb = [[nc.dram_tensor(f"outb{b}_{j}", [NCORES, D2, 64], BF)
             for j in range(NCH)] for b in range(B)]

    xt_r = xt.rearrange("(kc p) t -> p kc t", p=128)
    outp_r = outp.rearrange("(kc p) t -> p kc t", p=128)

    with tile.TileContext(nc) as tc:
        with (
            tc.tile_pool(name="const", bufs=1) as cpool,
            tc.tile_pool(name="w", bufs=1) as wpool,
            tc.tile_pool(name="xb", bufs=2) as xpool,
            tc.tile_pool(name="qk", bufs=2) as qkpool,
            tc.tile_pool(name="vstg", bufs=2) as vstgpool,
            tc.tile_pool(name="vh", bufs=2) as vhpool,
            tc.tile_pool(name="pt", bufs=5) as ptpool,
            tc.tile_pool(name="small", bufs=2) as smallpool,
            tc.tile_pool(name="obuf", bufs=2) as obufpool,
            tc.tile_pool(name="proj", bufs=4) as projpool,
            tc.tile_pool(name="otp", bufs=4) as otpool,
            tc.tile_pool(name="wps", bufs=2, space="PSUM") as wps,
            tc.tile_pool(name="s", bufs=2, space="PSUM") as sps,
            tc.tile_pool(name="o", bufs=1, space="PSUM") as ops,
        ):
            # ---- constants ----
            ident32 = cpool.tile([128, 128], F32)
            make_identity(nc, ident32[:])
            identb = cpool.tile([128, 128], BF)
            onesb = cpool.tile([128, HL * NTK], BF)
            # head-selector rows for the denominator broadcast matmuls;
            # single-partition tiles (partition starts must be 0/32/64/96)
            sel01 = cpool.tile([1, 256], BF)
            nc.gpsimd.memset(onesb[:], 1.0)
            nc.gpsimd.memset(sel01[:], 0.0)
            nc.gpsimd.memset(sel01[0:1, 0:64], 1.0)
            nc.gpsimd.memset(sel01[0:1, 192:256], 1.0)
            with nc.allow_low_precision(reason="bf16 operand staging"):
                nc.vector.tensor_copy(identb[:], ident32[:])

            # ---- state ----
            st = {}

            def load_xb_chunk(b, n):
                if ("xb", b) not in st:
                    st["xb", b] = xpool.tile([128, NKC, TB], BF, tag="xb",
                                             name=f"xb{b}")
                nc.sync.dma_start(
                    st["xb", b][:, :, 512 * n:512 * (n + 1)],
                    xt_r[:, :, TB * b + 512 * n:TB * b + 512 * (n + 1)])

            # ---- input + weight DMAs, most-urgent first ----
            # wq first, then the first chunk in 2-kc pieces so the first
            # qkv chain units start as soon as their slices land
            st["xb", 0] = xpool.tile([128, NKC, TB], BF, tag="xb",
                                     name="xb0")
            wq_sb = wpool.tile([128, NKC, D2], BF)
            wk_sb = wpool.tile([128, NKC, D2], BF)
            wv_sb = wpool.tile([128, NKC, D2], BF)
            nc.sync.dma_start(
                wq_sb[:], wq.rearrange("(kc p) m -> p kc m", p=128))
            for kc0 in range(0, NKC, 2):
                nc.sync.dma_start(
                    st["xb", 0][:, kc0:kc0 + 2, 0:512],
                    xt_r[:, kc0:kc0 + 2, 0:512])
            nc.sync.dma_start(
                wk_sb[:], wk.rearrange("(kc p) m -> p kc m", p=128))
            nc.sync.dma_start(
                wv_sb[:], wv.rearrange("(kc p) m -> p kc m", p=128))
            bqkv_sb = cpool.tile([D2, 3], F32)
            nc.sync.dma_start(bqkv_sb[:], bqkv)
            for n in range(1, NCH):
                load_xb_chunk(0, n)
            wp_sb = wpool.tile([128, NKC, C], BF)
            nc.sync.dma_start(
                wp_sb[:], wp.rearrange("(kc p) m -> p kc m", p=128))
            bp_sb = cpool.tile([128, NKC], F32)
            nc.sync.dma_start(bp_sb[:], bp)

            # ---- building blocks ----
            def qkv_alloc(b):
                st["qT", b] = qkpool.tile([D2, TB], BF, tag="qT",
                                          name=f"qT{b}")
                st["kT", b] = qkpool.tile([D2, TB], BF, tag="kT",
                                          name=f"kT{b}")
                st["vT", b] = vstgpool.tile([D2, TB], BF, tag="vT",
                                            name=f"vT{b}")

            def vh_alloc(b):
                vh = vhpool.tile([128, HL, NTK * 65], BF, tag="vh",
                                 name=f"vh{b}")
                st["vh", b] = vh
                with nc.allow_low_precision(reason="bf16 v ones"):
                    nc.vector.tensor_copy(
                        vh[:, :, 64::65],
                        onesb[:].rearrange("p (g t) -> p g t", g=HL))

            def qkv_chain_units(b, n, col):
                """Units (2 matmuls each) for one qkv projection chain."""
                w_sb = (wq_sb, wk_sb, wv_sb)[col]
                hold = {}

                def unit(kc0, b=b, n=n, col=col):
                    def f():
                        if kc0 == 0:
                            if n == 0 and col == 0:
                                qkv_alloc(b)
                            hold["ps"] = wps.tile([128, 512], F32, tag="w",
                                                  name=f"qkvps{b}_{n}_{col}")
                        ps = hold["ps"]
                        xbt = st["xb", b]
                        for kc in (kc0, kc0 + 1):
                            nc.tensor.matmul(
                                ps[:], w_sb[:, kc, :],
                                xbt[:, kc, 512 * n:512 * (n + 1)],
                                start=(kc == 0), stop=(kc == NKC - 1))
                        if kc0 == NKC - 2:
                            dst = (st["qT", b], st["kT", b],
                                   st["vT", b])[col]
                            with nc.allow_low_precision(reason="bf16 qkv"):
                                nc.vector.tensor_scalar_add(
                                    dst[:, 512 * n:512 * (n + 1)], ps[:],
                                    bqkv_sb[:, col:col + 1])
                    return f
                return [(unit(kc0), 426) for kc0 in range(0, NKC, 2)]

            def vtrans_unit(b, tk):
                def f():
                    if tk == 0:
                        vh_alloc(b)
                    vh, vT = st["vh", b], st["vT", b]
                    vt_ps = wps.tile([128, 128], BF, tag="w",
                                     name=f"vt{b}_{tk}")
                    nc.tensor.transpose(
                        vt_ps[:], vT[:, 128 * tk:128 * (tk + 1)], identb[:])
                    with nc.allow_low_precision(reason="bf16 v"):
                        nc.vector.tensor_copy(
                            vh[:, :, 65 * tk:65 * tk + 64],
                            vt_ps[:].rearrange("p (g d) -> p g d", g=HL))
                return f, 60

            def qkv_chunk_units(b, n):
                """All units for qkv chunk n of batch b (q,k,v chains; the
                v chain is followed by the transposes it unlocks)."""
                units = []
                for col in range(3):
                    units += qkv_chain_units(b, n, col)
                for tk in range(4 * n, 4 * n + 4):
                    units.append(vtrans_unit(b, tk))
                return units

            def proj_units(bb):
                """Out-projection of batch bb, one unit = 2 matmuls.
                Reads the per-j ot strips loaded during attn(bb)."""
                hold = {}

                def unit(mcol, s0, bb=bb):
                    def f():
                        if mcol == 0 and s0 == 0:
                            hold["psb"] = projpool.tile(
                                [128, NKC, PIECE], BF, tag="psb",
                                name=f"psb{bb}")
                        if s0 == 0:
                            hold["pp"] = wps.tile([128, PIECE], F32, tag="w",
                                                  name=f"pp{bb}_{mcol}")
                        pp, ot = hold["pp"], st["ot", bb]
                        for s8 in (s0, s0 + 1):
                            nc.tensor.matmul(
                                pp[:],
                                wp_sb[:, s8, 128 * mcol:128 * (mcol + 1)],
                                ot[:, s8], start=(s8 == 0),
                                stop=(s8 == NCORES - 1))
                        if s0 == NCORES - 2:
                            with nc.allow_low_precision(reason="bf16 out"):
                                nc.vector.tensor_scalar_add(
                                    hold["psb"][:, mcol, :], pp[:],
                                    bp_sb[:, mcol:mcol + 1])
                            if mcol == 5:
                                nc.sync.dma_start(
                                    outp_r[:, 0:6,
                                           PIECE * bb:PIECE * (bb + 1)],
                                    hold["psb"][:, 0:6, :])
                            if mcol == NKC - 1:
                                nc.sync.dma_start(
                                    outp_r[:, 6:8,
                                           PIECE * bb:PIECE * (bb + 1)],
                                    hold["psb"][:, 6:8, :])
                    return f
                return [(unit(mcol, s0), 214) for mcol in range(NKC)
                        for s0 in range(0, NCORES, 2)]

            class Fill:
                """Global queue of small PE work units. Units are emitted
                into the instruction stream between attention tiles so the
                PE never idles on the exp round-trip. Marks record queue
                positions that must be fully emitted before a dependent
                attention chunk runs (deadlock avoidance: the in-order PE
                queue may never wait on work emitted after it)."""

                def __init__(self):
                    self.q = []
                    self.i = 0
                    self.marks = {}

                def add(self, units, mark=None):
                    self.q += units
                    if mark is not None:
                        self.marks[mark] = len(self.q)

                def run_ns(self, budget):
                    while budget > 0 and self.i < len(self.q):
                        fn, est = self.q[self.i]
                        fn()
                        self.i += 1
                        budget -= est

                def drain_to(self, mark):
                    upto = self.marks.get(mark, 0)
                    while self.i < upto:
                        self.q[self.i][0]()
                        self.i += 1

                def drain(self):
                    while self.i < len(self.q):
                        self.q[self.i][0]()
                        self.i += 1

            def attn_j(b, j, fill, tile_fill):
                fill.drain_to(("qkv", b, j))
                qT, kT, vh = st["qT", b], st["kT", b], st["vh", b]
                obuf = st["obuf", b]
                if j == 0:
                    st["ot", b] = otpool.tile([128, NCORES, NCH, 64], BF,
                                              tag="ot", name=f"ot{b}")
                o_ps = ops.tile([65, 1024], F32, tag="o", name=f"o{b}_{j}")
                ktop = 4 * j + 4
                pend = []          # tks with emitted exp but not yet O

                def emit_o(tk):
                    m = tk - 4 * j
                    z = 128 * m if m > 0 else 0
                    pt = st["pt", tk]
                    for h in range(HL):
                        nc.tensor.matmul(
                            o_ps[0:65, 512 * h + z:512 * (h + 1)],
                            vh[:, h, 65 * tk:65 * (tk + 1)],
                            pt[:, 512 * h + z:512 * (h + 1)],
                            start=(tk == 0), stop=(tk == ktop - 1))

                for tk in range(ktop):
                    m = tk - 4 * j
                    z = 128 * m if m > 0 else 0
                    s_ps = sps.tile([128, 1024], F32, tag="s_ps")
                    for h in range(HL):
                        nc.tensor.matmul(
                            s_ps[:, 512 * h + z:512 * (h + 1)],
                            kT[64 * h:64 * (h + 1),
                               128 * tk:128 * (tk + 1)],
                            qT[64 * h:64 * (h + 1),
                               512 * j + z:512 * (j + 1)],
                            start=True, stop=True)
                    pt = ptpool.tile([128, 1024], BF, tag="pt")
                    st["pt", tk] = pt
                    with nc.allow_low_precision(reason="bf16 P"):
                        if z:
                            exp_src = s_ps[:].rearrange(
                                "p (g c) -> p g c", g=2)[:, :, z:]
                            exp_dst = pt[:].rearrange(
                                "p (g c) -> p g c", g=2)[:, :, z:]
                            nc.scalar.activation(exp_dst, exp_src, Exp)
                        else:
                            nc.scalar.activation(pt[:], s_ps[:], Exp)
                        if m >= 0:
                            # zero the upper triangle of the mixed 128-block
                            blk = pt[:].rearrange(
                                "p (g c) -> p g c", g=2)[:, :, z:z + 128]
                            nc.gpsimd.affine_select(
                                out=blk, in_=blk,
                                compare_op=mybir.AluOpType.is_ge, fill=0.0,
                                base=0, channel_multiplier=-1,
                                pattern=[[0, 2], [1, 128]],
                            )
                    fill.run_ns(tile_fill)
                    pend.append(tk)
                    if len(pend) > 4:
                        emit_o(pend.pop(0))
                for tk in pend:
                    emit_o(tk)

                # ---- normalization (both heads at once) ----
                r1 = smallpool.tile([1, 1024], BF, tag="r1")
                with nc.allow_low_precision(reason="softmax denom"):
                    for h in range(HL):
                        nc.vector.reciprocal(
                            r1[0:1, 512 * h:512 * h + 512],
                            o_ps[64:65, 512 * h:512 * h + 512])
                fill.run_ns(1200)
                rb_ps = wps.tile([128, 512], F32, tag="w", name=f"rb{b}_{j}")
                nc.tensor.matmul(rb_ps[:], sel01[0:1, 0:128],
                                 r1[0:1, 0:512], start=True, stop=False)
                nc.tensor.matmul(rb_ps[:], sel01[0:1, 128:256],
                                 r1[0:1, 512:1024], start=False, stop=True)
                rb_sb = smallpool.tile([128, 512], F32, tag="rb")
                nc.vector.tensor_copy(rb_sb[:], rb_ps[:])
                fill.run_ns(600)
                with nc.allow_low_precision(reason="bf16 O"):
                    for h in range(HL):
                        nc.vector.tensor_mul(
                            obuf[64 * h:64 * (h + 1),
                                 512 * j:512 * (j + 1)],
                            o_ps[0:64, 512 * h:512 * h + 512],
                            rb_sb[64 * h:64 * (h + 1), :])
                # ship this j-chunk: its 64-token strips are final, so the
                # AllToAll for this chunk fires immediately; the received
                # strips land in this batch's ot tile for the deferred proj.
                nc.sync.dma_start(
                    inb[b][j].ap().rearrange("u d p -> d u p"),
                    obuf[:, 512 * j:512 * (j + 1)].rearrange(
                        "d (u p) -> d u p", u=NCORES))
                if sim:
                    # stand-in with comparable cost for the cost-model sim
                    nc.sync.dma_start(outb[b][j].ap(), inb[b][j].ap())
                else:
                    nc.gpsimd.collective_compute(
                        "AllToAll", mybir.AluOpType.bypass,
                        replica_groups=[list(range(NCORES))],
                        ins=[inb[b][j].ap().opt()],
                        outs=[outb[b][j].ap().opt()],
                    )
                nc.sync.dma_start(
                    st["ot", b][:, :, j, :],
                    outb[b][j].ap().rearrange("u d p -> d u p"))

            # ---- main schedule ----
            # batch 0 prelude: chunk 0 of qkv(0) emitted directly
            for u, _ in qkv_chunk_units(0, 0):
                u()
            fill = Fill()
            for b in range(B):
                st["obuf", b] = obufpool.tile([128, TB], BF, tag="obuf",
                                              name=f"obuf{b}")
                if b == 0:
                    for n in range(1, NCH):
                        fill.add(qkv_chunk_units(0, n), mark=("qkv", 0, n))
                if b >= 2:
                    fill.add(proj_units(b - 2))
                if b + 1 < B:
                    for n in range(NCH):
                        load_xb_chunk(b + 1, n)
                    for n in range(NCH):
                        fill.add(qkv_chunk_units(b + 1, n),
                                 mark=("qkv", b + 1, n))
                for j in range(NCH):
                    attn_j(b, j, fill, 400)
            fill.add(proj_units(B - 2))
            fill.add(proj_units(B - 1))
            fill.drain()
    nc.compile()
    return nc


def _get_nc():
    if "nc" not in _CACHE:
        _CACHE["nc"] = _build()
    return _CACHE["nc"]


def kernel(x, W_qkv, b_qkv, W_proj, b_proj):
    x = np.asarray(x, dtype=np.float32)
    W_qkv = np.asarray(W_qkv, dtype=np.float32)
    b_qkv = np.asarray(b_qkv, dtype=np.float32)
    W_proj = np.asarray(W_proj, dtype=np.float32)
    b_proj = np.asarray(b_proj, dtype=np.float32)

    bf = ml_dtypes.bfloat16
    scale = 1.0 / np.sqrt(HD)
    xt = np.ascontiguousarray(x.reshape(B * T, C).T.astype(bf))  # [C, B*T]
    wp = np.ascontiguousarray(W_proj.astype(bf))                 # [C, C]
    bp = np.ascontiguousarray(b_proj.reshape(NKC, 128).T)        # [128, 8]

    qw = W_qkv[:, 0:C]
    kw = W_qkv[:, C:2 * C]
    vw = W_qkv[:, 2 * C:3 * C]
    qb, kb, vb = b_qkv[0:C], b_qkv[C:2 * C], b_qkv[2 * C:3 * C]

    in_maps = []
    for c in range(NCORES):
        cols = slice(2 * c * HD, (2 * c + 2) * HD)  # this core's 128 dims
        bq = np.stack([qb[cols] * scale, kb[cols], vb[cols]], axis=1)  # [128,3]
        in_maps.append({
            "xt": xt,
            "wq": np.ascontiguousarray((qw[:, cols] * scale).astype(bf)),
            "wk": np.ascontiguousarray(kw[:, cols].astype(bf)),
            "wv": np.ascontiguousarray(vw[:, cols].astype(bf)),
            "wp": wp,
            "bqkv": np.ascontiguousarray(bq, dtype=np.float32),
            "bp": np.ascontiguousarray(bp, dtype=np.float32),
        })

    nc = _get_nc()
    _CACHE["last_in_maps"] = in_maps
    res = run_bass_kernel_spmd(nc, in_maps, core_ids=list(range(NCORES)))

    # outp[r]: [C, B*PIECE] (cols: b-major, then rank r's 4 strips of 64
    # tokens, one per 512-token j-chunk: global token = 512j + 64r + t)
    allo = np.stack([np.asarray(res.results[c]["outp"])
                     for c in range(NCORES)])
    allo = allo.reshape(NCORES, C, B, NCH, 64).astype(np.float32)
    out = allo.transpose(2, 3, 0, 4, 1).reshape(B, T, C)
    return np.ascontiguousarray(out)


# revision 56
# speedup vs baseline: 1.5881x; 1.0037x over previous
"""Causal self-attention (B=4, T=2048, C=1024, H=16) on 8 trn2 NeuronCores.

Sharding: head-pair parallel. Core c owns heads {2c, 2c+1} for all 4 batches.
 - host: x is pre-transposed to xT [C, B*T] in bf16; W_qkv is pre-sliced per
   core into one wqkv [C, 384] bf16 (2 heads x 64 for each of q/k/v, softmax
   scale folded into the q part), W_proj bf16 and f32 biases broadcast.
 - device per core: qkv projections as bf16 matmuls producing qT/kT/vT
   [d2, T] bf16 (d on partitions); vT is PE-transposed per 128-tile into a
   combined per-head v [T, 65] layout (ones column appended for the softmax
   denominator).
 - attention in S^T orientation: S^T[tk, tq] = kT.T@qT tiles [128, 512], both
   heads side by side in one PSUM tile [128, 1024]. Softmax without
   max-subtraction (|S| <= ~20, safe in f32 PSUM): P^T = exp(S^T) on ScalarE
   (PSUM->SBUF, bf16). Causal masking of the diagonal 128x128 sub-block is a
   post-exp affine_select (fill 0) on Pool; fully-masked columns are skipped
   via the z offset. O-matmul lhsT = [v_h | ones] (M=65) yields both
   O^T[d, tq] and the denominator row l in one pass. Normalize via
   reciprocal + two K=1 broadcast matmuls + DVE multiply into a per-batch
   O^T accumulator [128, T] bf16.
 - resharding in 64-token strips: rank r's piece of tq-chunk j is tokens
   [512j + 64r, 512j + 64r + 64), so each chunk's AllToAll (128 KB) fires as
   soon as that chunk's softmax completes and the received strips accumulate
   into a per-batch ot tile; the column-parallel out-projection with fused
   bias is deferred two batches and produces out^T bf16; host reassembles.
 - scheduling: all non-attention PE work (qkv chains of later batches, v
   transposes, deferred out-projections) is chopped into ~0.2-0.4us filler
   units in a global queue and emitted between attention tiles under a
   PE-nanosecond budget, so the PE never idles on the exp round-trip or the
   collectives; per-chunk marks force-drain a batch's qkv units before the
   attention that reads them (the in-order PE queue must never wait on work
   emitted after it). S->exp->O is software-pipelined (O lags S by 3 tiles).
"""
import numpy as np
import ml_dtypes
import concourse.bacc as bacc
import concourse.mybir as mybir
import concourse.tile as tile
from concourse.bass_utils import run_bass_kernel_spmd
from concourse.masks import make_identity

F32 = mybir.dt.float32
F32R = mybir.dt.float32r
BF = mybir.dt.bfloat16
Exp = mybir.ActivationFunctionType.Exp

NCORES = 8
B, T, C, H = 4, 2048, 1024, 16
HD = C // H          # 64
HL = H // NCORES     # 2 heads per core
D2 = HL * HD         # 128 rows of local head-pair dims
TB = T               # tokens per batch
NKC = C // 128       # 8 contraction chunks
NCH = TB // 512      # 4 tq chunks per batch
NTK = TB // 128      # 16 tk tiles per batch
PIECE = TB // NCORES  # 256 tokens per (batch, core) piece after AllToAll

_CACHE = {}


def _build(sim=False):
    nc = bacc.Bacc("TRN2", target_bir_lowering=False, debug=False,
                   num_devices=1 if sim else NCORES)
    xt = nc.dram_tensor("xt", [C, B * T], BF, kind="ExternalInput").ap()
    wqkv = nc.dram_tensor("wqkv", [C, 3 * D2], BF, kind="ExternalInput").ap()
    wp = nc.dram_tensor("wp", [C, C], BF, kind="ExternalInput").ap()
    bqkv = nc.dram_tensor("bqkv", [D2, 3], F32, kind="ExternalInput").ap()
    bp = nc.dram_tensor("bp", [128, NKC], F32, kind="ExternalInput").ap()
    outp = nc.dram_tensor("outp", [C, B * PIECE], BF,
                          kind="ExternalOutput").ap()

    # per-(batch, j-chunk) 64-token strips: rank r's piece of chunk j is
    # tokens [512j + 64r, 512j + 64r + 64) — uniform across ranks, so each
    # chunk's AllToAll can fire as soon as that chunk's softmax completes.
    inb = [[nc.dram_tensor(f"inb{b}_{j}", [NCORES, D2, 64], BF)
            for j in range(NCH)] for b in range(B)]
    outb = [[nc.dram_tensor(f"outb{b}_{j}", [NCORES, D2, 64], BF)
             for j in range(NCH)] for b in range(B)]

    xt_r = xt.rearrange("(kc p) t -> p kc t", p=128)
    outp_r = outp.rearrange("(kc p) t -> p kc t", p=128)

    with tile.TileContext(nc) as tc:
        with (
            tc.tile_pool(name="const", bufs=1) as cpool,
            tc.tile_pool(name="w", bufs=1) as wpool,
            tc.tile_pool(name="xb", bufs=2) as xpool,
            tc.tile_pool(name="qk", bufs=2) as qkpool,
            tc.tile_pool(name="vstg", bufs=2) as vstgpool,
            tc.tile_pool(name="vh", bufs=2) as vhpool,
            tc.tile_pool(name="pt", bufs=5) as ptpool,
            tc.tile_pool(name="small", bufs=2) as smallpool,
            tc.tile_pool(name="obuf", bufs=2) as obufpool,
            tc.tile_pool(name="proj", bufs=4) as projpool,
            tc.tile_pool(name="otp", bufs=4) as otpool,
            tc.tile_pool(name="wps", bufs=2, space="PSUM") as wps,
            tc.tile_pool(name="s", bufs=2, space="PSUM") as sps,
            tc.tile_pool(name="o", bufs=1, space="PSUM") as ops,
        ):
            # ---- constants ----
            ident32 = cpool.tile([128, 128], F32)
            make_identity(nc, ident32[:])
            identb = cpool.tile([128, 128], BF)
            onesb = cpool.tile([128, HL * NTK], BF)
            # head-selector rows for the denominator broadcast matmuls;
            # single-partition tiles (partition starts must be 0/32/64/96)
            sel01 = cpool.tile([1, 256], BF)
            nc.gpsimd.memset(onesb[:], 1.0)
            nc.gpsimd.memset(sel01[:], 0.0)
            nc.gpsimd.memset(sel01[0:1, 0:64], 1.0)
            nc.gpsimd.memset(sel01[0:1, 192:256], 1.0)
            with nc.allow_low_precision(reason="bf16 operand staging"):
                nc.vector.tensor_copy(identb[:], ident32[:])

            # ---- state ----
            st = {}

            def load_xb_chunk(b, n):
                if ("xb", b) not in st:
                    st["xb", b] = xpool.tile([128, NKC, TB], BF, tag="xb",
                                             name=f"xb{b}")
                nc.sync.dma_start(
                    st["xb", b][:, :, 512 * n:512 * (n + 1)],
                    xt_r[:, :, TB * b + 512 * n:TB * b + 512 * (n + 1)])

            # ---- input + weight DMAs, most-urgent first ----
            # combined wqkv rows are 768B (>=512B: no sub-512 DMA latency
            # penalty, unlike three separate 256B-row loads); stream it in
            # 2-kc pieces interleaved with the first x chunk's pieces so
            # the first qkv chain units start as soon as their slices land
            st["xb", 0] = xpool.tile([128, NKC, TB], BF, tag="xb",
                                     name="xb0")
            wqkv_sb = wpool.tile([128, NKC, 3 * D2], BF)
            wqkv_r = wqkv.rearrange("(kc p) m -> p kc m", p=128)
            nc.sync.dma_start(wqkv_sb[:, 0:2, :], wqkv_r[:, 0:2, :])
            nc.sync.dma_start(st["xb", 0][:, 0:2, 0:512],
                              xt_r[:, 0:2, 0:512])
            nc.sync.dma_start(wqkv_sb[:, 2:4, :], wqkv_r[:, 2:4, :])
            nc.sync.dma_start(st["xb", 0][:, 2:4, 0:512],
                              xt_r[:, 2:4, 0:512])
            nc.sync.dma_start(wqkv_sb[:, 4:8, :], wqkv_r[:, 4:8, :])
            nc.sync.dma_start(st["xb", 0][:, 4:8, 0:512],
                              xt_r[:, 4:8, 0:512])
            bqkv_sb = cpool.tile([D2, 3], F32)
            nc.sync.dma_start(bqkv_sb[:], bqkv)
            for n in range(1, NCH):
                load_xb_chunk(0, n)
            wp_sb = wpool.tile([128, NKC, C], BF)
            nc.sync.dma_start(
                wp_sb[:], wp.rearrange("(kc p) m -> p kc m", p=128))
            bp_sb = cpool.tile([128, NKC], F32)
            nc.sync.dma_start(bp_sb[:], bp)

            # ---- building blocks ----
            def qkv_alloc(b):
                st["qT", b] = qkpool.tile([D2, TB], BF, tag="qT",
                                          name=f"qT{b}")
                st["kT", b] = qkpool.tile([D2, TB], BF, tag="kT",
                                          name=f"kT{b}")
                st["vT", b] = vstgpool.tile([D2, TB], BF, tag="vT",
                                            name=f"vT{b}")

            def vh_alloc(b):
                vh = vhpool.tile([128, HL, NTK * 65], BF, tag="vh",
                                 name=f"vh{b}")
                st["vh", b] = vh
                with nc.allow_low_precision(reason="bf16 v ones"):
                    nc.vector.tensor_copy(
                        vh[:, :, 64::65],
                        onesb[:].rearrange("p (g t) -> p g t", g=HL))

            def qkv_chain_units(b, n, col):
                """Units (2 matmuls each) for one qkv projection chain."""
                hold = {}

                def unit(kc0, b=b, n=n, col=col):
                    def f():
                        if kc0 == 0:
                            if n == 0 and col == 0:
                                qkv_alloc(b)
                            hold["ps"] = wps.tile([128, 512], F32, tag="w",
                                                  name=f"qkvps{b}_{n}_{col}")
                        ps = hold["ps"]
                        xbt = st["xb", b]
                        for kc in (kc0, kc0 + 1):
                            nc.tensor.matmul(
                                ps[:],
                                wqkv_sb[:, kc, D2 * col:D2 * (col + 1)],
                                xbt[:, kc, 512 * n:512 * (n + 1)],
                                start=(kc == 0), stop=(kc == NKC - 1))
                        if kc0 == NKC - 2:
                            dst = (st["qT", b], st["kT", b],
                                   st["vT", b])[col]
                            with nc.allow_low_precision(reason="bf16 qkv"):
                                nc.vector.tensor_scalar_add(
                                    dst[:, 512 * n:512 * (n + 1)], ps[:],
                                    bqkv_sb[:, col:col + 1])
                    return f
                return [(unit(kc0), 426) for kc0 in range(0, NKC, 2)]

            def vtrans_unit(b, tk):
                def f():
                    if tk == 0:
                        vh_alloc(b)
                    vh, vT = st["vh", b], st["vT", b]
                    vt_ps = wps.tile([128, 128], BF, tag="w",
                                     name=f"vt{b}_{tk}")
                    nc.tensor.transpose(
                        vt_ps[:], vT[:, 128 * tk:128 * (tk + 1)], identb[:])
                    with nc.allow_low_precision(reason="bf16 v"):
                        nc.vector.tensor_copy(
                            vh[:, :, 65 * tk:65 * tk + 64],
                            vt_ps[:].rearrange("p (g d) -> p g d", g=HL))
                return f, 60

            def qkv_chunk_units(b, n):
                """All units for qkv chunk n of batch b (q,k,v chains; the
                v chain is followed by the transposes it unlocks)."""
                units = []
                for col in range(3):
                    units += qkv_chain_units(b, n, col)
                for tk in range(4 * n, 4 * n + 4):
                    units.append(vtrans_unit(b, tk))
                return units

            def proj_units(bb):
                """Out-projection of batch bb, one unit = 2 matmuls.
                Reads the per-j ot strips loaded during attn(bb)."""
                hold = {}

                def unit(mcol, s0, bb=bb):
                    def f():
                        if mcol == 0 and s0 == 0:
                            hold["psb"] = projpool.tile(
                                [128, NKC, PIECE], BF, tag="psb",
                                name=f"psb{bb}")
                        if s0 == 0:
                            hold["pp"] = wps.tile([128, PIECE], F32, tag="w",
                                                  name=f"pp{bb}_{mcol}")
                        pp, ot = hold["pp"], st["ot", bb]
                        for s8 in (s0, s0 + 1):
                            nc.tensor.matmul(
                                pp[:],
                                wp_sb[:, s8, 128 * mcol:128 * (mcol + 1)],
                                ot[:, s8], start=(s8 == 0),
                                stop=(s8 == NCORES - 1))
                        if s0 == NCORES - 2:
                            with nc.allow_low_precision(reason="bf16 out"):
                                nc.vector.tensor_scalar_add(
                                    hold["psb"][:, mcol, :], pp[:],
                                    bp_sb[:, mcol:mcol + 1])
                            if mcol == 5:
                                nc.sync.dma_start(
                                    outp_r[:, 0:6,
                                           PIECE * bb:PIECE * (bb + 1)],
                                    hold["psb"][:, 0:6, :])
                            if mcol == NKC - 1:
                                nc.sync.dma_start(
                                    outp_r[:, 6:8,
                                           PIECE * bb:PIECE * (bb + 1)],
                                    hold["psb"][:, 6:8, :])
                    return f
                return [(unit(mcol, s0), 214) for mcol in range(NKC)
                        for s0 in range(0, NCORES, 2)]

            class Fill:
                """Global queue of small PE work units. Units are emitted
                into the instruction stream between attention tiles so the
                PE never idles on the exp round-trip. Marks record queue
                positions that must be fully emitted before a dependent
                attention chunk runs (deadlock avoidance: the in-order PE
                queue may never wait on work emitted after it)."""

                def __init__(self):
                    self.q = []
                    self.i = 0
                    self.marks = {}

                def add(self, units, mark=None):
                    self.q += units
                    if mark is not None:
                        self.marks[mark] = len(self.q)

                def run_ns(self, budget):
                    while budget > 0 and self.i < len(self.q):
                        fn, est = self.q[self.i]
                        fn()
                        self.i += 1
                        budget -= est

                def drain_to(self, mark):
                    upto = self.marks.get(mark, 0)
                    while self.i < upto:
                        self.q[self.i][0]()
                        self.i += 1

                def drain(self):
                    while self.i < len(self.q):
                        self.q[self.i][0]()
                        self.i += 1

            def attn_j(b, j, fill, tile_fill):
                fill.drain_to(("qkv", b, j))
                qT, kT, vh = st["qT", b], st["kT", b], st["vh", b]
                obuf = st["obuf", b]
                if j == 0:
                    st["ot", b] = otpool.tile([128, NCORES, NCH, 64], BF,
                                              tag="ot", name=f"ot{b}")
                o_ps = ops.tile([65, 1024], F32, tag="o", name=f"o{b}_{j}")
                ktop = 4 * j + 4
                pend = []          # tks with emitted exp but not yet O

                def emit_o(tk):
                    m = tk - 4 * j
                    z = 128 * m if m > 0 else 0
                    pt = st["pt", tk]
                    for h in range(HL):
                        nc.tensor.matmul(
                            o_ps[0:65, 512 * h + z:512 * (h + 1)],
                            vh[:, h, 65 * tk:65 * (tk + 1)],
                            pt[:, 512 * h + z:512 * (h + 1)],
                            start=(tk == 0), stop=(tk == ktop - 1))

                for tk in range(ktop):
                    m = tk - 4 * j
                    z = 128 * m if m > 0 else 0
                    s_ps = sps.tile([128, 1024], F32, tag="s_ps")
                    for h in range(HL):
                        nc.tensor.matmul(
                            s_ps[:, 512 * h + z:512 * (h + 1)],
                            kT[64 * h:64 * (h + 1),
                               128 * tk:128 * (tk + 1)],
                            qT[64 * h:64 * (h + 1),
                               512 * j + z:512 * (j + 1)],
                            start=True, stop=True)
                    pt = ptpool.tile([128, 1024], BF, tag="pt")
                    st["pt", tk] = pt
                    with nc.allow_low_precision(reason="bf16 P"):
                        if z:
                            exp_src = s_ps[:].rearrange(
                                "p (g c) -> p g c", g=2)[:, :, z:]
                            exp_dst = pt[:].rearrange(
                                "p (g c) -> p g c", g=2)[:, :, z:]
                            nc.scalar.activation(exp_dst, exp_src, Exp)
                        else:
                            nc.scalar.activation(pt[:], s_ps[:], Exp)
                        if m >= 0:
                            # zero the upper triangle of the mixed 128-block
                            blk = pt[:].rearrange(
                                "p (g c) -> p g c", g=2)[:, :, z:z + 128]
                            nc.gpsimd.affine_select(
                                out=blk, in_=blk,
                                compare_op=mybir.AluOpType.is_ge, fill=0.0,
                                base=0, channel_multiplier=-1,
                                pattern=[[0, 2], [1, 128]],
                            )
                    fill.run_ns(tile_fill)
                    pend.append(tk)
                    if len(pend) > 4:
                        emit_o(pend.pop(0))
                for tk in pend:
                    emit_o(tk)

                # ---- normalization (both heads at once) ----
                r1 = smallpool.tile([1, 1024], BF, tag="r1")
                with nc.allow_low_precision(reason="softmax denom"):
                    for h in range(HL):
                        nc.vector.reciprocal(
                            r1[0:1, 512 * h:512 * h + 512],
                            o_ps[64:65, 512 * h:512 * h + 512])
                fill.run_ns(1200)
                rb_ps = wps.tile([128, 512], F32, tag="w", name=f"rb{b}_{j}")
                nc.tensor.matmul(rb_ps[:], sel01[0:1, 0:128],
                                 r1[0:1, 0:512], start=True, stop=False)
                nc.tensor.matmul(rb_ps[:], sel01[0:1, 128:256],
                                 r1[0:1, 512:1024], start=False, stop=True)
                rb_sb = smallpool.tile([128, 512], F32, tag="rb")
                nc.vector.tensor_copy(rb_sb[:], rb_ps[:])
                fill.run_ns(600)
                with nc.allow_low_precision(reason="bf16 O"):
                    for h in range(HL):
                        nc.vector.tensor_mul(
                            obuf[64 * h:64 * (h + 1),
                                 512 * j:512 * (j + 1)],
                            o_ps[0:64, 512 * h:512 * h + 512],
                            rb_sb[64 * h:64 * (h + 1), :])
                # ship this j-chunk: its 64-token strips are final, so the
                # AllToAll for this chunk fires immediately; the received
                # strips land in this batch's ot tile for the deferred proj.
                nc.sync.dma_start(
                    inb[b][j].ap().rearrange("u d p -> d u p"),
                    obuf[:, 512 * j:512 * (j + 1)].rearrange(
                        "d (u p) -> d u p", u=NCORES))
                if sim:
                    # stand-in with comparable cost for the cost-model sim
                    nc.sync.dma_start(outb[b][j].ap(), inb[b][j].ap())
                else:
                    nc.gpsimd.collective_compute(
                        "AllToAll", mybir.AluOpType.bypass,
                        replica_groups=[list(range(NCORES))],
                        ins=[inb[b][j].ap().opt()],
                        outs=[outb[b][j].ap().opt()],
                    )
                nc.sync.dma_start(
                    st["ot", b][:, :, j, :],
                    outb[b][j].ap().rearrange("u d p -> d u p"))

            # ---- main schedule ----
            # batch 0 prelude: chunk 0 of qkv(0) emitted directly
            for u, _ in qkv_chunk_units(0, 0):
                u()
            fill = Fill()
            for b in range(B):
                st["obuf", b] = obufpool.tile([128, TB], BF, tag="obuf",
                                              name=f"obuf{b}")
                if b == 0:
                    for n in range(1, NCH):
                        fill.add(qkv_chunk_units(0, n), mark=("qkv", 0, n))
                if b >= 2:
                    fill.add(proj_units(b - 2))
                if b + 1 < B:
                    for n in range(NCH):
                        load_xb_chunk(b + 1, n)
                    for n in range(NCH):
                        fill.add(qkv_chunk_units(b + 1, n),
                                 mark=("qkv", b + 1, n))
                for j in range(NCH):
                    attn_j(b, j, fill, 400)
            fill.add(proj_units(B - 2))
            fill.add(proj_units(B - 1))
            fill.drain()
    nc.compile()
    return nc


def _get_nc():
    if "nc" not in _CACHE:
        _CACHE["nc"] = _build()
    return _CACHE["nc"]


def kernel(x, W_qkv, b_qkv, W_proj, b_proj):
    x = np.asarray(x, dtype=np.float32)
    W_qkv = np.asarray(W_qkv, dtype=np.float32)
    b_qkv = np.asarray(b_qkv, dtype=np.float32)
    W_proj = np.asarray(W_proj, dtype=np.float32)
    b_proj = np.asarray(b_proj, dtype=np.float32)

    bf = ml_dtypes.bfloat16
    scale = 1.0 / np.sqrt(HD)
    xt = np.ascontiguousarray(x.reshape(B * T, C).T.astype(bf))  # [C, B*T]
    wp = np.ascontiguousarray(W_proj.astype(bf))                 # [C, C]
    bp = np.ascontiguousarray(b_proj.reshape(NKC, 128).T)        # [128, 8]

    qw = W_qkv[:, 0:C]
    kw = W_qkv[:, C:2 * C]
    vw = W_qkv[:, 2 * C:3 * C]
    qb, kb, vb = b_qkv[0:C], b_qkv[C:2 * C], b_qkv[2 * C:3 * C]

    in_maps = []
    for c in range(NCORES):
        cols = slice(2 * c * HD, (2 * c + 2) * HD)  # this core's 128 dims
        bq = np.stack([qb[cols] * scale, kb[cols], vb[cols]], axis=1)  # [128,3]
        in_maps.append({
            "xt": xt,
            "wqkv": np.ascontiguousarray(np.concatenate(
                [qw[:, cols] * scale, kw[:, cols], vw[:, cols]],
                axis=1).astype(bf)),
            "wp": wp,
            "bqkv": np.ascontiguousarray(bq, dtype=np.float32),
            "bp": np.ascontiguousarray(bp, dtype=np.float32),
        })

    nc = _get_nc()
    _CACHE["last_in_maps"] = in_maps
    res = run_bass_kernel_spmd(nc, in_maps, core_ids=list(range(NCORES)))

    # outp[r]: [C, B*PIECE] (cols: b-major, then rank r's 4 strips of 64
    # tokens, one per 512-token j-chunk: global token = 512j + 64r + t)
    allo = np.stack([np.asarray(res.results[c]["outp"])
                     for c in range(NCORES)])
    allo = allo.reshape(NCORES, C, B, NCH, 64).astype(np.float32)
    out = allo.transpose(2, 3, 0, 4, 1).reshape(B, T, C)
    return np.ascontiguousarray(out)


# revision 65
# speedup vs baseline: 1.6229x; 1.0219x over previous
"""Causal self-attention (B=4, T=2048, C=1024, H=16) on 8 trn2 NeuronCores.

Sharding: head-pair parallel. Core c owns heads {2c, 2c+1} for all 4 batches.
 - host: x is pre-transposed to xT [C, B*T] in bf16; W_qkv is pre-sliced per
   core into one wqkv [C, 384] bf16 (2 heads x 64 for each of q/k/v, softmax
   scale folded into the q part), W_proj bf16 and f32 biases broadcast.
 - device per core: qkv projections as bf16 matmuls producing qT/kT/vT
   [d2, T] bf16 (d on partitions); vT is PE-transposed per 128-tile into a
   combined per-head v [T, 65] layout (ones column appended for the softmax
   denominator).
 - attention in S^T orientation: S^T[tk, tq] = kT.T@qT tiles [128, 512], both
   heads side by side in one PSUM tile [128, 1024]. Softmax without
   max-subtraction (|S| <= ~20, safe in f32 PSUM): P^T = exp(S^T) on ScalarE
   (PSUM->SBUF, bf16). Causal masking of the diagonal 128x128 sub-block is a
   post-exp affine_select (fill 0) on Pool; fully-masked columns are skipped
   via the z offset. O-matmul lhsT = [v_h | ones] (M=65) yields both
   O^T[d, tq] and the denominator row l in one pass. Normalize via
   reciprocal + two K=1 broadcast matmuls + DVE multiply into a per-batch
   O^T accumulator [128, T] bf16.
 - resharding in 64-token strips: rank r's piece of tq-chunk j is tokens
   [512j + 64r, 512j + 64r + 64), so each chunk's AllToAll (128 KB) fires as
   soon as that chunk's softmax completes and the received strips accumulate
   into a per-batch ot tile; the column-parallel out-projection with fused
   bias is deferred two batches and produces out^T bf16; host reassembles.
 - scheduling: all non-attention PE work (qkv chains of later batches, v
   transposes, deferred out-projections) is chopped into ~0.2-0.4us filler
   units in a global queue and emitted between attention tiles under a
   PE-nanosecond budget, so the PE never idles on the exp round-trip or the
   collectives; per-chunk marks force-drain a batch's qkv units before the
   attention that reads them (the in-order PE queue must never wait on work
   emitted after it). S->exp->O is software-pipelined (O lags S by 3 tiles).
"""
import numpy as np
import ml_dtypes
import concourse.bacc as bacc
import concourse.mybir as mybir
import concourse.tile as tile
from concourse.bass_utils import run_bass_kernel_spmd
from concourse.masks import make_identity

F32 = mybir.dt.float32
F32R = mybir.dt.float32r
BF = mybir.dt.bfloat16
Exp = mybir.ActivationFunctionType.Exp

NCORES = 8
B, T, C, H = 4, 2048, 1024, 16
HD = C // H          # 64
HL = H // NCORES     # 2 heads per core
D2 = HL * HD         # 128 rows of local head-pair dims
TB = T               # tokens per batch
NKC = C // 128       # 8 contraction chunks
NCH = TB // 512      # 4 tq chunks per batch
NTK = TB // 128      # 16 tk tiles per batch
PIECE = TB // NCORES  # 256 tokens per (batch, core) piece after AllToAll

_CACHE = {}


def _build(sim=False):
    nc = bacc.Bacc("TRN2", target_bir_lowering=False, debug=False,
                   num_devices=1 if sim else NCORES)
    xt = nc.dram_tensor("xt", [C, B * T], BF, kind="ExternalInput").ap()
    wqkv = nc.dram_tensor("wqkv", [C, 3 * D2], BF, kind="ExternalInput").ap()
    wp = nc.dram_tensor("wp", [C, C], BF, kind="ExternalInput").ap()
    bqkv = nc.dram_tensor("bqkv", [D2, 3], F32, kind="ExternalInput").ap()
    bp = nc.dram_tensor("bp", [128, NKC], F32, kind="ExternalInput").ap()
    outp = nc.dram_tensor("outp", [C, B * PIECE], BF,
                          kind="ExternalOutput").ap()

    # per-(batch, j-chunk) 64-token strips: rank r's piece of chunk j is
    # tokens [512j + 64r, 512j + 64r + 64) — uniform across ranks, so each
    # chunk's AllToAll can fire as soon as that chunk's softmax completes.
    inb = [[nc.dram_tensor(f"inb{b}_{j}", [NCORES, D2, 64], BF)
            for j in range(NCH)] for b in range(B)]
    outb = [[nc.dram_tensor(f"outb{b}_{j}", [NCORES, D2, 64], BF)
             for j in range(NCH)] for b in range(B)]

    xt_r = xt.rearrange("(kc p) t -> p kc t", p=128)
    outp_r = outp.rearrange("(kc p) t -> p kc t", p=128)

    with tile.TileContext(nc) as tc:
        with (
            tc.tile_pool(name="const", bufs=1) as cpool,
            tc.tile_pool(name="w", bufs=1) as wpool,
            tc.tile_pool(name="xb", bufs=2) as xpool,
            tc.tile_pool(name="qk", bufs=2) as qkpool,
            tc.tile_pool(name="vstg", bufs=2) as vstgpool,
            tc.tile_pool(name="vh", bufs=2) as vhpool,
            tc.tile_pool(name="pt", bufs=5) as ptpool,
            tc.tile_pool(name="small", bufs=2) as smallpool,
            tc.tile_pool(name="obuf", bufs=2) as obufpool,
            tc.tile_pool(name="proj", bufs=4) as projpool,
            tc.tile_pool(name="otp", bufs=4) as otpool,
            tc.tile_pool(name="wps", bufs=2, space="PSUM") as wps,
            tc.tile_pool(name="s", bufs=2, space="PSUM") as sps,
            tc.tile_pool(name="o", bufs=1, space="PSUM") as ops,
        ):
            # ---- constants ----
            ident32 = cpool.tile([128, 128], F32)
            make_identity(nc, ident32[:])
            identb = cpool.tile([128, 128], BF)
            onesb = cpool.tile([128, HL * NTK], BF)
            # head-selector rows for the denominator broadcast matmuls;
            # single-partition tiles (partition starts must be 0/32/64/96)
            sel01 = cpool.tile([1, 256], BF)
            nc.gpsimd.memset(onesb[:], 1.0)
            nc.gpsimd.memset(sel01[:], 0.0)
            nc.gpsimd.memset(sel01[0:1, 0:64], 1.0)
            nc.gpsimd.memset(sel01[0:1, 192:256], 1.0)
            with nc.allow_low_precision(reason="bf16 operand staging"):
                nc.vector.tensor_copy(identb[:], ident32[:])

            def warmup(n, rhs):
                # dependency-free matmuls into a scratch PSUM slot: keep the
                # PE busy through a DMA-wait block so the instructions
                # dispatched during it are costed at full pstate, not cold
                jps = wps.tile([128, rhs.shape[-1]], F32, tag="w",
                               name="jps")
                for i in range(n):
                    nc.tensor.matmul(jps[:], identb[:], rhs,
                                     start=True, stop=True)

            warmup(16, identb[:])

            # ---- state ----
            st = {}

            def load_xb_chunk(b, n):
                if ("xb", b) not in st:
                    st["xb", b] = xpool.tile([128, NKC, TB], BF, tag="xb",
                                             name=f"xb{b}")
                nc.sync.dma_start(
                    st["xb", b][:, :, 512 * n:512 * (n + 1)],
                    xt_r[:, :, TB * b + 512 * n:TB * b + 512 * (n + 1)])

            # ---- input + weight DMAs, most-urgent first ----
            # combined wqkv rows are 768B (>=512B: no sub-512 DMA latency
            # penalty, unlike three separate 256B-row loads); stream it in
            # 2-kc pieces interleaved with the first x chunk's pieces so
            # the first qkv chain units start as soon as their slices land
            st["xb", 0] = xpool.tile([128, NKC, TB], BF, tag="xb",
                                     name="xb0")
            wqkv_sb = wpool.tile([128, NKC, 3 * D2], BF)
            wqkv_r = wqkv.rearrange("(kc p) m -> p kc m", p=128)
            nc.sync.dma_start(wqkv_sb[:, 0:2, :], wqkv_r[:, 0:2, :])
            nc.sync.dma_start(st["xb", 0][:, 0:2, 0:512],
                              xt_r[:, 0:2, 0:512])
            nc.sync.dma_start(wqkv_sb[:, 2:4, :], wqkv_r[:, 2:4, :])
            nc.sync.dma_start(st["xb", 0][:, 2:4, 0:512],
                              xt_r[:, 2:4, 0:512])
            nc.sync.dma_start(wqkv_sb[:, 4:8, :], wqkv_r[:, 4:8, :])
            nc.sync.dma_start(st["xb", 0][:, 4:8, 0:512],
                              xt_r[:, 4:8, 0:512])
            bqkv_sb = cpool.tile([D2, 3], F32)
            nc.sync.dma_start(bqkv_sb[:], bqkv)
            for n in range(1, NCH):
                load_xb_chunk(0, n)
            wp_sb = wpool.tile([128, NKC, C], BF)
            nc.sync.dma_start(
                wp_sb[:], wp.rearrange("(kc p) m -> p kc m", p=128))
            bp_sb = cpool.tile([128, NKC], F32)
            nc.sync.dma_start(bp_sb[:], bp)

            # ---- building blocks ----
            def qkv_alloc(b):
                st["qT", b] = qkpool.tile([D2, TB], BF, tag="qT",
                                          name=f"qT{b}")
                st["kT", b] = qkpool.tile([D2, TB], BF, tag="kT",
                                          name=f"kT{b}")
                st["vT", b] = vstgpool.tile([D2, TB], BF, tag="vT",
                                            name=f"vT{b}")

            def vh_alloc(b):
                vh = vhpool.tile([128, HL, NTK * 65], BF, tag="vh",
                                 name=f"vh{b}")
                st["vh", b] = vh
                with nc.allow_low_precision(reason="bf16 v ones"):
                    nc.vector.tensor_copy(
                        vh[:, :, 64::65],
                        onesb[:].rearrange("p (g t) -> p g t", g=HL))

            def qkv_chain_units(b, n, col):
                """Units (2 matmuls each) for one qkv projection chain."""
                hold = {}

                def unit(kc0, b=b, n=n, col=col):
                    def f():
                        if kc0 == 0:
                            if n == 0 and col == 0:
                                qkv_alloc(b)
                            hold["ps"] = wps.tile([128, 512], F32, tag="w",
                                                  name=f"qkvps{b}_{n}_{col}")
                        ps = hold["ps"]
                        xbt = st["xb", b]
                        for kc in (kc0, kc0 + 1):
                            nc.tensor.matmul(
                                ps[:],
                                wqkv_sb[:, kc, D2 * col:D2 * (col + 1)],
                                xbt[:, kc, 512 * n:512 * (n + 1)],
                                start=(kc == 0), stop=(kc == NKC - 1))
                        if kc0 == NKC - 2:
                            dst = (st["qT", b], st["kT", b],
                                   st["vT", b])[col]
                            with nc.allow_low_precision(reason="bf16 qkv"):
                                nc.vector.tensor_scalar_add(
                                    dst[:, 512 * n:512 * (n + 1)], ps[:],
                                    bqkv_sb[:, col:col + 1])
                    return f
                return [(unit(kc0), 426) for kc0 in range(0, NKC, 2)]

            def vtrans_unit(b, tk):
                def f():
                    if tk == 0:
                        vh_alloc(b)
                    vh, vT = st["vh", b], st["vT", b]
                    vt_ps = wps.tile([128, 128], BF, tag="w",
                                     name=f"vt{b}_{tk}")
                    nc.tensor.transpose(
                        vt_ps[:], vT[:, 128 * tk:128 * (tk + 1)], identb[:])
                    with nc.allow_low_precision(reason="bf16 v"):
                        nc.vector.tensor_copy(
                            vh[:, :, 65 * tk:65 * tk + 64],
                            vt_ps[:].rearrange("p (g d) -> p g d", g=HL))
                return f, 60

            def qkv_chunk_units(b, n):
                """All units for qkv chunk n of batch b (q,k,v chains; the
                v chain is followed by the transposes it unlocks)."""
                units = []
                for col in range(3):
                    units += qkv_chain_units(b, n, col)
                for tk in range(4 * n, 4 * n + 4):
                    units.append(vtrans_unit(b, tk))
                return units

            def proj_units(bb):
                """Out-projection of batch bb, one unit = 2 matmuls.
                Reads the per-j ot strips loaded during attn(bb)."""
                hold = {}

                def unit(mcol, s0, bb=bb):
                    def f():
                        if mcol == 0 and s0 == 0:
                            hold["psb"] = projpool.tile(
                                [128, NKC, PIECE], BF, tag="psb",
                                name=f"psb{bb}")
                        if s0 == 0:
                            hold["pp"] = wps.tile([128, PIECE], F32, tag="w",
                                                  name=f"pp{bb}_{mcol}")
                        pp, ot = hold["pp"], st["ot", bb]
                        for s8 in (s0, s0 + 1):
                            nc.tensor.matmul(
                                pp[:],
                                wp_sb[:, s8, 128 * mcol:128 * (mcol + 1)],
                                ot[:, s8], start=(s8 == 0),
                                stop=(s8 == NCORES - 1))
                        if s0 == NCORES - 2:
                            with nc.allow_low_precision(reason="bf16 out"):
                                nc.vector.tensor_scalar_add(
                                    hold["psb"][:, mcol, :], pp[:],
                                    bp_sb[:, mcol:mcol + 1])
                            if mcol == 5:
                                nc.sync.dma_start(
                                    outp_r[:, 0:6,
                                           PIECE * bb:PIECE * (bb + 1)],
                                    hold["psb"][:, 0:6, :])
                            if mcol == NKC - 1:
                                nc.sync.dma_start(
                                    outp_r[:, 6:8,
                                           PIECE * bb:PIECE * (bb + 1)],
                                    hold["psb"][:, 6:8, :])
                    return f
                return [(unit(mcol, s0), 214) for mcol in range(NKC)
                        for s0 in range(0, NCORES, 2)]

            class Fill:
                """Global queue of small PE work units. Units are emitted
                into the instruction stream between attention tiles so the
                PE never idles on the exp round-trip. Marks record queue
                positions that must be fully emitted before a dependent
                attention chunk runs (deadlock avoidance: the in-order PE
                queue may never wait on work emitted after it)."""

                def __init__(self):
                    self.q = []
                    self.i = 0
                    self.marks = {}

                def add(self, units, mark=None):
                    self.q += units
                    if mark is not None:
                        self.marks[mark] = len(self.q)

                def run_ns(self, budget):
                    while budget > 0 and self.i < len(self.q):
                        fn, est = self.q[self.i]
                        fn()
                        self.i += 1
                        budget -= est

                def drain_to(self, mark):
                    upto = self.marks.get(mark, 0)
                    while self.i < upto:
                        self.q[self.i][0]()
                        self.i += 1

                def drain(self):
                    while self.i < len(self.q):
                        self.q[self.i][0]()
                        self.i += 1

            def attn_j(b, j, fill, tile_fill):
                fill.drain_to(("qkv", b, j))
                qT, kT, vh = st["qT", b], st["kT", b], st["vh", b]
                obuf = st["obuf", b]
                if j == 0:
                    st["ot", b] = otpool.tile([128, NCORES, NCH, 64], BF,
                                              tag="ot", name=f"ot{b}")
                o_ps = ops.tile([65, 1024], F32, tag="o", name=f"o{b}_{j}")
                ktop = 4 * j + 4
                pend = []          # tks with emitted exp but not yet O

                def emit_o(tk):
                    m = tk - 4 * j
                    z = 128 * m if m > 0 else 0
                    pt = st["pt", tk]
                    for h in range(HL):
                        nc.tensor.matmul(
                            o_ps[0:65, 512 * h + z:512 * (h + 1)],
                            vh[:, h, 65 * tk:65 * (tk + 1)],
                            pt[:, 512 * h + z:512 * (h + 1)],
                            start=(tk == 0), stop=(tk == ktop - 1))

                for tk in range(ktop):
                    m = tk - 4 * j
                    z = 128 * m if m > 0 else 0
                    s_ps = sps.tile([128, 1024], F32, tag="s_ps")
                    for h in range(HL):
                        nc.tensor.matmul(
                            s_ps[:, 512 * h + z:512 * (h + 1)],
                            kT[64 * h:64 * (h + 1),
                               128 * tk:128 * (tk + 1)],
                            qT[64 * h:64 * (h + 1),
                               512 * j + z:512 * (j + 1)],
                            start=True, stop=True)
                    pt = ptpool.tile([128, 1024], BF, tag="pt")
                    st["pt", tk] = pt
                    with nc.allow_low_precision(reason="bf16 P"):
                        if z:
                            exp_src = s_ps[:].rearrange(
                                "p (g c) -> p g c", g=2)[:, :, z:]
                            exp_dst = pt[:].rearrange(
                                "p (g c) -> p g c", g=2)[:, :, z:]
                            nc.scalar.activation(exp_dst, exp_src, Exp)
                        else:
                            nc.scalar.activation(pt[:], s_ps[:], Exp)
                        if m >= 0:
                            # zero the upper triangle of the mixed 128-block
                            blk = pt[:].rearrange(
                                "p (g c) -> p g c", g=2)[:, :, z:z + 128]
                            nc.gpsimd.affine_select(
                                out=blk, in_=blk,
                                compare_op=mybir.AluOpType.is_ge, fill=0.0,
                                base=0, channel_multiplier=-1,
                                pattern=[[0, 2], [1, 128]],
                            )
                    fill.run_ns(tile_fill)
                    pend.append(tk)
                    if len(pend) > 4:
                        emit_o(pend.pop(0))
                for tk in pend:
                    emit_o(tk)

                # ---- normalization (both heads at once) ----
                r1 = smallpool.tile([1, 1024], BF, tag="r1")
                with nc.allow_low_precision(reason="softmax denom"):
                    for h in range(HL):
                        nc.vector.reciprocal(
                            r1[0:1, 512 * h:512 * h + 512],
                            o_ps[64:65, 512 * h:512 * h + 512])
                fill.run_ns(1200)
                rb_ps = wps.tile([128, 512], F32, tag="w", name=f"rb{b}_{j}")
                nc.tensor.matmul(rb_ps[:], sel01[0:1, 0:128],
                                 r1[0:1, 0:512], start=True, stop=False)
                nc.tensor.matmul(rb_ps[:], sel01[0:1, 128:256],
                                 r1[0:1, 512:1024], start=False, stop=True)
                rb_sb = smallpool.tile([128, 512], F32, tag="rb")
                nc.vector.tensor_copy(rb_sb[:], rb_ps[:])
                fill.run_ns(600)
                with nc.allow_low_precision(reason="bf16 O"):
                    for h in range(HL):
                        nc.vector.tensor_mul(
                            obuf[64 * h:64 * (h + 1),
                                 512 * j:512 * (j + 1)],
                            o_ps[0:64, 512 * h:512 * h + 512],
                            rb_sb[64 * h:64 * (h + 1), :])
                # ship this j-chunk: its 64-token strips are final, so the
                # AllToAll for this chunk fires immediately; the received
                # strips land in this batch's ot tile for the deferred proj.
                nc.sync.dma_start(
                    inb[b][j].ap().rearrange("u d p -> d u p"),
                    obuf[:, 512 * j:512 * (j + 1)].rearrange(
                        "d (u p) -> d u p", u=NCORES))
                if sim:
                    # stand-in with comparable cost for the cost-model sim
                    nc.sync.dma_start(outb[b][j].ap(), inb[b][j].ap())
                else:
                    nc.gpsimd.collective_compute(
                        "AllToAll", mybir.AluOpType.bypass,
                        replica_groups=[list(range(NCORES))],
                        ins=[inb[b][j].ap().opt()],
                        outs=[outb[b][j].ap().opt()],
                    )
                nc.sync.dma_start(
                    st["ot", b][:, :, j, :],
                    outb[b][j].ap().rearrange("u d p -> d u p"))

            # ---- main schedule ----
            # batch 0 prelude: chunk 0 of qkv(0) emitted directly
            for u, _ in qkv_chunk_units(0, 0):
                u()
            fill = Fill()
            for b in range(B):
                st["obuf", b] = obufpool.tile([128, TB], BF, tag="obuf",
                                              name=f"obuf{b}")
                if b == 0:
                    for n in range(1, NCH):
                        fill.add(qkv_chunk_units(0, n), mark=("qkv", 0, n))
                if b >= 2:
                    fill.add(proj_units(b - 2))
                if b + 1 < B:
                    for n in range(NCH):
                        load_xb_chunk(b + 1, n)
                    for n in range(NCH):
                        fill.add(qkv_chunk_units(b + 1, n),
                                 mark=("qkv", b + 1, n))
                for j in range(NCH):
                    attn_j(b, j, fill, 400)
            fill.add(proj_units(B - 2))
            fill.add([(lambda: warmup(22, identb[:]), 0)])
            fill.add(proj_units(B - 1))
            fill.drain()
    nc.compile()
    return nc


def _get_nc():
    if "nc" not in _CACHE:
        _CACHE["nc"] = _build()
    return _CACHE["nc"]


def kernel(x, W_qkv, b_qkv, W_proj, b_proj):
    x = np.asarray(x, dtype=np.float32)
    W_qkv = np.asarray(W_qkv, dtype=np.float32)
    b_qkv = np.asarray(b_qkv, dtype=np.float32)
    W_proj = np.asarray(W_proj, dtype=np.float32)
    b_proj = np.asarray(b_proj, dtype=np.float32)

    bf = ml_dtypes.bfloat16
    scale = 1.0 / np.sqrt(HD)
    xt = np.ascontiguousarray(x.reshape(B * T, C).T.astype(bf))  # [C, B*T]
    wp = np.ascontiguousarray(W_proj.astype(bf))                 # [C, C]
    bp = np.ascontiguousarray(b_proj.reshape(NKC, 128).T)        # [128, 8]

    qw = W_qkv[:, 0:C]
    kw = W_qkv[:, C:2 * C]
    vw = W_qkv[:, 2 * C:3 * C]
    qb, kb, vb = b_qkv[0:C], b_qkv[C:2 * C], b_qkv[2 * C:3 * C]

    in_maps = []
    for c in range(NCORES):
        cols = slice(2 * c * HD, (2 * c + 2) * HD)  # this core's 128 dims
        bq = np.stack([qb[cols] * scale, kb[cols], vb[cols]], axis=1)  # [128,3]
        in_maps.append({
            "xt": xt,
            "wqkv": np.ascontiguousarray(np.concatenate(
                [qw[:, cols] * scale, kw[:, cols], vw[:, cols]],
                axis=1).astype(bf)),
            "wp": wp,
            "bqkv": np.ascontiguousarray(bq, dtype=np.float32),
            "bp": np.ascontiguousarray(bp, dtype=np.float32),
        })

    nc = _get_nc()
    _CACHE["last_in_maps"] = in_maps
    res = run_bass_kernel_spmd(nc, in_maps, core_ids=list(range(NCORES)))

    # outp[r]: [C, B*PIECE] (cols: b-major, then rank r's 4 strips of 64
    # tokens, one per 512-token j-chunk: global token = 512j + 64r + t)
    allo = np.stack([np.asarray(res.results[c]["outp"])
                     for c in range(NCORES)])
    allo = allo.reshape(NCORES, C, B, NCH, 64).astype(np.float32)
    out = allo.transpose(2, 3, 0, 4, 1).reshape(B, T, C)
    return np.ascontiguousarray(out)
